# revision 1
# baseline (speedup 1.0000x reference)
"""Complex transformer block (LN->attn->LN->MLP, complex arithmetic) on 8 TRN2 cores.

Sharding: core c handles (batch b = c//2, sequence half = c%2). No collectives:
each core computes K/V over the full 1024-token sequence of its batch (the only
duplicated work) and queries/MLP over its own 512 tokens.

Layout: activations are feature-major [feature partition-blocks, tokens].
Complex tensors are realified as separate real/imag feature planes. LayerNorm
is fused into the following matmul: per-token stats (mu_r, mu_i, std) are
appended as 3 extra contraction rows with matching weight columns, and the
per-token rstd is applied by the PSUM-eviction multiply. Attention scores are
computed transposed ([t2, t1]) so softmax sums reduce via ones-matmuls, and V
is produced pre-transposed by swapping matmul operands. All matmuls run as
float32r (E8M11) at full PE rate; data entering matmuls is pre-rounded on the
host or written as float32r by the producing DVE/ACT op.
"""
import sys
sys.path.insert(0, "/opt/trn_rl_repo")

from contextlib import ExitStack

import numpy as np

import concourse.bacc as bacc
import concourse.bass as bass
import concourse.mybir as mybir
import concourse.tile as tile
from concourse.bass_utils import run_bass_kernel_spmd

# Prefer the table set that covers the whole softmax chain (square+ln+exp)
# so the greedy act-table-load pass doesn't thrash sets on every block.
_orig_get_tables = bacc.get_activation_tables


def _reordered_tables(arch):
    # Keep canonical order/indices (walrus resolves act_func_set_id by
    # position) but blank every set except the two we want, so the greedy
    # table-load pass can't thrash between sets per softmax block.
    t = _orig_get_tables(arch)
    keep = {"natural_log_exp_and_others", "gelu_and_others"}
    return {k: (v if k in keep else set()) for k, v in t.items()}


bacc.get_activation_tables = _reordered_tables

dt = mybir.dt
AF = mybir.ActivationFunctionType
ALU = mybir.AluOpType

B, N, C, H, DH, HID = 4, 1024, 768, 12, 64, 3072
NCORES = 8
OWN = 512          # tokens per core
KB = C // 128      # 6 feature pblocks per plane
SCALE = DH ** -0.5
EPS = 1e-5
MC = 256           # MLP token chunk


def round_fp32r(x):
    b = np.ascontiguousarray(x, dtype=np.float32).view(np.uint32)
    lsb = (b >> np.uint32(12)) & np.uint32(1)
    return ((b + np.uint32(0x7FF) + lsb) & np.uint32(0xFFFFF000)).view(np.float32)


# --------------------------------------------------------------------------
# device program
# --------------------------------------------------------------------------

def build_nc(debug=False):
    nc = bacc.Bacc(trn_type="TRN2", target_bir_lowering=False)
    f32 = dt.float32
    f32r = dt.float32r

    # ---- DRAM I/O ----
    x_r = nc.dram_tensor("x_r", [12, 128, N], f32r, kind="ExternalInput")
    x_own = nc.dram_tensor("x_own", [12, 128, OWN], f32, kind="ExternalInput")
    w_qkv = nc.dram_tensor("w_qkv", [H, 12, 128, 384], f32r, kind="ExternalInput")
    w_qkv_s = nc.dram_tensor("w_qkv_s", [H, 4, 384], f32r, kind="ExternalInput")
    w_v = nc.dram_tensor("w_v", [12, 128, 1536], f32r, kind="ExternalInput")
    w_v_s = nc.dram_tensor("w_v_s", [4, 1536], f32r, kind="ExternalInput")
    w_proj = nc.dram_tensor("w_proj", [12, 12, 128, 128], f32r, kind="ExternalInput")
    w_proj_s = nc.dram_tensor("w_proj_s", [12, 4, 128], f32r, kind="ExternalInput")
    w_fc1r = nc.dram_tensor("w_fc1r", [24, 6, 128, 128], f32r, kind="ExternalInput")
    w_fc1i = nc.dram_tensor("w_fc1i", [24, 6, 128, 128], f32r, kind="ExternalInput")
    w_fc1_s = nc.dram_tensor("w_fc1_s", [24, 4, 256], f32r, kind="ExternalInput")
    w_fc2r = nc.dram_tensor("w_fc2r", [6, 24, 128, 128], f32r, kind="ExternalInput")
    w_fc2i = nc.dram_tensor("w_fc2i", [6, 24, 128, 128], f32r, kind="ExternalInput")
    w_fc2_s = nc.dram_tensor("w_fc2_s", [6, 4, 256], f32r, kind="ExternalInput")
    ones_col = nc.dram_tensor("ones_col", [128, 1], f32r, kind="ExternalInput")
    ones_row = nc.dram_tensor("ones_row", [1, 128], f32r, kind="ExternalInput")
    ones_ab = nc.dram_tensor("ones_ab", [128, 4], f32r, kind="ExternalInput")
    # ones_ab cols: [1/C, 0] (A: xr-plane mu), [0, 1/C] (B: xi-plane mu)
    ones_s = nc.dram_tensor("ones_s", [128, 1], f32r, kind="ExternalInput")  # 1/C
    stat_one = nc.dram_tensor("stat_one", [4, OWN], f32r, kind="ExternalInput")

    out_fm = nc.dram_tensor("out_fm", [12, 128, OWN], f32, kind="ExternalOutput")
    dbg = {}
    if debug:
        for nm, shp in [
            ("d_stat1", [4, N]), ("d_q1", [128, OWN]), ("d_k", [128, N]),
            ("d_vt", [128, 8, 256]), ("d_exp", [128, OWN]), ("d_den", [1, OWN]),
            ("d_attn", [128, OWN]), ("d_r1", [128, OWN]), ("d_stat2", [4, OWN]),
            ("d_h", [128, MC]), ("d_S", [1, 512]), ("d_var", [1, 512]),
            ("d_mufl", [1, 2, 512]), ("d_sq0", [128, 512]),
            ("d_rstdT", [128, 8]),
        ]:
            dbg[nm] = nc.dram_tensor(nm, shp, f32, kind="ExternalOutput")

    with tile.TileContext(nc) as tc, ExitStack() as top:
        consts = top.enter_context(tc.tile_pool(name="consts", bufs=1))
        t_ones_col = consts.tile([128, 1], f32r)
        t_ones_row = consts.tile([1, 128], f32r)
        t_ones_ab = consts.tile([128, 4], f32r)
        t_ones_s = consts.tile([128, 1], f32r)
        t_stat_one = consts.tile([4, OWN], f32r)
        t_eps = consts.tile([1, 1], f32)
        nc.sync.dma_start(t_ones_col[:], ones_col[:])
        nc.sync.dma_start(t_ones_row[:], ones_row[:])
        nc.sync.dma_start(t_ones_ab[:], ones_ab[:])
        nc.sync.dma_start(t_ones_s[:], ones_s[:])
        nc.sync.dma_start(t_stat_one[:], stat_one[:])
        nc.vector.memset(t_eps[:], EPS)

        poolR1 = top.enter_context(tc.tile_pool(name="poolR1", bufs=1))
        xr1 = poolR1.tile([128, 12, OWN], f32, name="xr1")
        nc.sync.dma_start(xr1[:], x_own[:].rearrange("b p n -> p b n"))

        with ExitStack() as es_x:
            poolX = es_x.enter_context(tc.tile_pool(name="poolX", bufs=1))
            xr = poolX.tile([128, 12, N], f32r, name="xr")
            pdram = es_x.enter_context(
                tc.tile_pool(name="pdram", bufs=1, space="DRAM"))
            rstd_dram = pdram.tile([1, N], f32, name="rstd_dram")
            stat1 = poolX.tile([4, N], f32r, name="stat1")
            rstd_bc1 = poolX.tile([128, N], f32, name="rstd_bc1")
            rstdT = poolX.tile([128, 8], f32, name="rstdT")
            for kb in range(12):
                nc.sync.dma_start(xr[:, kb, :], x_r[kb])

            # ---------------- phase A: LN1 stats over full sequence --------
            with ExitStack() as es_a:
                pa = es_a.enter_context(tc.tile_pool(name="pa_sb", bufs=12))
                pa_ps = es_a.enter_context(
                    tc.tile_pool(name="pa_ps", bufs=2, space="PSUM"))
                pa_sc = es_a.enter_context(tc.tile_pool(name="pa_sc", bufs=2))
                sqs = []
                for kb in range(12):
                    sq = pa.tile([128, N], f32r, tag="sq", name=f"sq{kb}")
                    nc.scalar.activation(sq[:], xr[:, kb, :], AF.Square)
                    sqs.append(sq)
                for ch in range(2):
                    sl = slice(ch * 512, ch * 512 + 512)
                    mu_ps = pa_ps.tile([2, 512], f32, tag="mu", name=f"mu{ch}")
                    s_ps = pa_ps.tile([1, 512], f32, tag="s", name=f"s{ch}")
                    for kb in range(12):
                        lhs = t_ones_ab[:, 0:2] if kb < 6 else t_ones_ab[:, 2:4]
                        nc.tensor.matmul(mu_ps[:], lhs, xr[:, kb, sl],
                                         start=(kb == 0), stop=(kb == 11))
                        nc.tensor.matmul(s_ps[:], t_ones_s[:], sqs[kb][:, sl],
                                         start=(kb == 0), stop=(kb == 11))
                    # var = S - mu_r^2 - mu_i^2 ; std = exp(.5 ln(var+eps))
                    mu_sb = pa_sc.tile([2, 512], f32, tag="musb", name=f"musb{ch}")
                    mu_fl = pa_sc.tile([1, 2, 512], f32, tag="mufl", name=f"mufl{ch}")
                    var = pa_sc.tile([1, 512], f32, tag="var", name=f"var{ch}")
                    lnv = pa_sc.tile([1, 512], f32, tag="lnv", name=f"lnv{ch}")
                    nc.vector.tensor_copy(mu_sb[:], mu_ps[:])
                    nc.sync.dma_start(mu_fl[:, 0, :], mu_sb[0:1, :])
                    nc.sync.dma_start(mu_fl[:, 1, :], mu_sb[1:2, :])
                    sq_mu = pa_sc.tile([1, 2, 512], f32, tag="sqmu", name=f"sqmu{ch}")
                    nc.vector.tensor_tensor(sq_mu[:], mu_fl[:], mu_fl[:],
                                            op=ALU.mult)
                    nc.vector.tensor_tensor(var[:], s_ps[:], sq_mu[:, 0, :],
                                            op=ALU.subtract)
                    nc.vector.tensor_tensor(var[:], var[:], sq_mu[:, 1, :],
                                            op=ALU.subtract)
                    nc.scalar.activation(lnv[:], var[:], AF.Ln, bias=t_eps[:])
                    if debug and ch == 0:
                        nc.sync.dma_start(dbg["d_var"][:], var[:])
                        nc.sync.dma_start(dbg["d_mufl"][:], mu_fl[:])
                        nc.sync.dma_start(dbg["d_sq0"][:],
                                          sqs[0][:, 0:512].bitcast(f32))
                        s_sb_dbg = pa_sc.tile([1, 512], f32, tag="sdbg",
                                              name="sdbg")
                        nc.vector.tensor_copy(s_sb_dbg[:], s_ps[:])
                        nc.sync.dma_start(dbg["d_S"][:], s_sb_dbg[:])
                    # stats rows: 0=mu_r 1=mu_i 2=std
                    nc.vector.tensor_copy(stat1[0:2, sl], mu_sb[:])
                    std_row = pa_sc.tile([1, 512], f32r, tag="stdr", name=f"stdr{ch}")
                    nc.scalar.activation(std_row[:], lnv[:], AF.Exp, scale=0.5)
                    nc.sync.dma_start(stat1[2:3, sl], std_row[:])
                    rstd_row = pa_sc.tile([1, 512], f32r, tag="rst", name=f"rst{ch}")
                    nc.scalar.activation(rstd_row[:], lnv[:], AF.Exp, scale=-0.5)
                    nc.sync.dma_start(rstd_dram[:, sl], rstd_row[:].bitcast(f32))
                    bcast = bass.AP(tensor=rstd_dram.tensor,
                                    offset=rstd_dram[:, sl].offset,
                                    ap=[[0, 128]] + rstd_dram[:, sl].ap[1:])
                    nc.sync.dma_start(rstd_bc1[:, sl], bcast)
                # rstd transposed: rstdT[p, t2b] = rstd[t2b*128 + p]
                nc.sync.dma_start(
                    rstdT[:],
                    rstd_dram[:].rearrange("o (b p) -> (o p) b", p=128))
                if debug:
                    nc.sync.dma_start(dbg["d_stat1"][:],
                                      stat1[:].bitcast(f32))
                    nc.sync.dma_start(dbg["d_rstdT"][:], rstdT[:])

            # ---------------- phase BC: qkv + attention per head ----------
            es_attn = ExitStack()
            attnp = es_attn.enter_context(tc.tile_pool(name="attnp", bufs=1))
            attn = attnp.tile([128, 12, OWN], f32r, name="attn")
            es_b = ExitStack()
            pq = es_b.enter_context(tc.tile_pool(name="pq", bufs=1))
            pk = es_b.enter_context(tc.tile_pool(name="pk", bufs=1))
            pvt = es_b.enter_context(tc.tile_pool(name="pvt", bufs=2))
            pwv = es_b.enter_context(tc.tile_pool(name="pwv", bufs=1))
            pwq = es_b.enter_context(tc.tile_pool(name="pwq", bufs=2))
            pet = es_b.enter_context(tc.tile_pool(name="pet", bufs=6))
            psc = es_b.enter_context(tc.tile_pool(name="psc", bufs=6))
            prd = es_b.enter_context(tc.tile_pool(name="prd", bufs=2))
            ps_rot = es_b.enter_context(
                tc.tile_pool(name="ps_rot", bufs=6, space="PSUM"))
            ps_acc = es_b.enter_context(
                tc.tile_pool(name="ps_acc", bufs=2, space="PSUM"))
            pdram_rd = es_b.enter_context(
                tc.tile_pool(name="pdram_rd", bufs=2, space="DRAM"))
            vt_pair = None
            et_fifo = []
            acc_ps = {}
            LAG = 4

            def emit_avden(ent):
                h2, t2b2, et2, vt2 = ent
                slot2 = h2 % 2
                if t2b2 == 0:
                    acc_ps[h2] = (
                        ps_acc.tile([128, OWN], f32, tag="av", name=f"av{h2}",
                                    bufs=1),
                        ps_acc.tile([1, OWN], f32, tag="den", name=f"den{h2}",
                                    bufs=1),
                    )
                av2, den2 = acc_ps[h2]
                nc.tensor.matmul(den2[:], t_ones_col[:], et2[:],
                                 start=(t2b2 == 0), stop=(t2b2 == 7))
                dsl2 = slice(slot2 * 128, slot2 * 128 + 128)
                nc.tensor.matmul(av2[:], vt2[:, t2b2, dsl2], et2[:],
                                 start=(t2b2 == 0), stop=(t2b2 == 7))
                if t2b2 == 7:
                    den_sb = prd.tile([1, OWN], f32, tag="den_sb",
                                      name=f"dsb{h2}", bufs=1)
                    rd = prd.tile([1, OWN], f32, tag="rd", name=f"rd{h2}",
                                  bufs=1)
                    nc.vector.tensor_copy(den_sb[:], den2[:])
                    nc.vector.reciprocal(rd[:], den_sb[:])
                    rd_dram = pdram_rd.tile([1, OWN], f32, tag="rdd",
                                            name=f"rdd{h2}")
                    nc.sync.dma_start(rd_dram[:], rd[:])
                    rd_bc = prd.tile([128, OWN], f32, tag="rd_bc",
                                     name=f"rdbc{h2}", bufs=1)
                    rd_bcast_ap = bass.AP(tensor=rd_dram.tensor,
                                          offset=rd_dram[:].offset,
                                          ap=[[0, 128]] + rd_dram[:].ap[1:])
                    nc.sync.dma_start(rd_bc[:], rd_bcast_ap)
                    nc.vector.tensor_tensor(attn[:, h2, :], av2[:], rd_bc[:],
                                            op=ALU.mult)
                    del acc_ps[h2]
                    if debug and h2 == 0:
                        nc.sync.dma_start(dbg["d_den"][:], den_sb[:])
                        nc.sync.dma_start(dbg["d_attn"][:],
                                          attn[:, 0, :].bitcast(f32))

            for h in range(H):
                pair, slot = divmod(h, 2)
                # qkv for head h: q1=[q_r;-q_i], q3=[q_i;q_r], k=[k_r;k_i]
                q_t = pq.tile([128, 2, OWN], f32r, tag="q", name=f"q{h}")
                k_t = pk.tile([128, N], f32r, tag="k", name=f"k{h}")
                wqs_t = pwq.tile([4, 384], f32r, tag="wqs", name=f"wqs{h}")
                nc.sync.dma_start(wqs_t[:], w_qkv_s[h])
                q1_ps = ps_rot.tile([128, OWN], f32, tag="rot", name=f"q1ps{h}")
                q3_ps = ps_rot.tile([128, OWN], f32, tag="rot", name=f"q3ps{h}")
                for kb in range(12):
                    wq_t = pwq.tile([128, 256], f32r, tag="wq",
                                    name=f"wq{h}_{kb}")
                    nc.sync.dma_start(wq_t[:], w_qkv[h, kb, :, 0:256])
                    st = (kb == 0)
                    nc.tensor.matmul(q1_ps[:], wq_t[:, 0:128],
                                     xr[:, kb, 0:OWN], start=st, stop=False)
                    nc.tensor.matmul(q3_ps[:], wq_t[:, 128:256],
                                     xr[:, kb, 0:OWN], start=st, stop=False)
                nc.tensor.matmul(q1_ps[:], wqs_t[:, 0:128], stat1[:, 0:OWN],
                                 start=False, stop=True)
                nc.tensor.matmul(q3_ps[:], wqs_t[:, 128:256], stat1[:, 0:OWN],
                                 start=False, stop=True)
                nc.vector.tensor_tensor(q_t[:, 0, :], q1_ps[:],
                                        rstd_bc1[:, 0:OWN], op=ALU.mult)
                nc.vector.tensor_tensor(q_t[:, 1, :], q3_ps[:],
                                        rstd_bc1[:, 0:OWN], op=ALU.mult)
                k0_ps = ps_rot.tile([128, 512], f32, tag="rot", name=f"k0ps{h}")
                k1_ps = ps_rot.tile([128, 512], f32, tag="rot", name=f"k1ps{h}")
                for kb in range(12):
                    wk_t = pwq.tile([128, 128], f32r, tag="wk",
                                    name=f"wk{h}_{kb}")
                    nc.sync.dma_start(wk_t[:], w_qkv[h, kb, :, 256:384])
                    st = (kb == 0)
                    nc.tensor.matmul(k0_ps[:], wk_t[:],
                                     xr[:, kb, 0:512], start=st, stop=False)
                    nc.tensor.matmul(k1_ps[:], wk_t[:],
                                     xr[:, kb, 512:N], start=st, stop=False)
                nc.tensor.matmul(k0_ps[:], wqs_t[:, 256:384], stat1[:, 0:512],
                                 start=False, stop=True)
                nc.tensor.matmul(k1_ps[:], wqs_t[:, 256:384], stat1[:, 512:N],
                                 start=False, stop=True)
                nc.vector.tensor_tensor(k_t[:, 0:512], k0_ps[:],
                                        rstd_bc1[:, 0:512], op=ALU.mult)
                nc.vector.tensor_tensor(k_t[:, 512:N], k1_ps[:],
                                        rstd_bc1[:, 512:N], op=ALU.mult)
                if debug and h == 0:
                    nc.sync.dma_start(dbg["d_q1"][:],
                                      q_t[:, 0, :].bitcast(f32))
                    nc.sync.dma_start(dbg["d_k"][:], k_t[:].bitcast(f32))
                if slot == 0:
                    # V^T for this head pair: [t2, d] via swapped operands
                    wv_t = pwv.tile([128, 12, 256], f32r, tag="wv",
                                    name=f"wv{pair}")
                    wv_s = pwv.tile([4, 256], f32r, tag="wvs",
                                    name=f"wvs{pair}")
                    csl = slice(pair * 256, pair * 256 + 256)
                    nc.sync.dma_start(wv_t[:], w_v[:, :, csl]
                                      .rearrange("b p n -> p b n"))
                    nc.sync.dma_start(wv_s[:], w_v_s[:, csl])
                    vt_pair = pvt.tile([128, 8, 256], f32r, tag="vt",
                                       name=f"vt{pair}")
                    for t2b in range(8):
                        t2s = slice(t2b * 128, t2b * 128 + 128)
                        vt_ps = ps_rot.tile([128, 256], f32, tag="rot",
                                            name=f"vtps{pair}_{t2b}")
                        for kb in range(12):
                            nc.tensor.matmul(vt_ps[:], xr[:, kb, t2s],
                                             wv_t[:, kb, :],
                                             start=(kb == 0), stop=False)
                        nc.tensor.matmul(vt_ps[:], stat1[:, t2s], wv_s[:],
                                         start=False, stop=True)
                        nc.vector.tensor_scalar(
                            vt_pair[:, t2b, :], vt_ps[:],
                            rstdT[:, t2b:t2b + 1], None, op0=ALU.mult)
                    if debug and pair == 0:
                        nc.sync.dma_start(dbg["d_vt"][:],
                                          vt_pair[:].bitcast(f32))
                # scores + exp chain; den/av matmuls lag by LAG blocks
                for t2b in range(8):
                    t2s = slice(t2b * 128, t2b * 128 + 128)
                    sr_ps = ps_rot.tile([128, OWN], f32, tag="rot",
                                        name=f"sr{h}_{t2b}")
                    si_ps = ps_rot.tile([128, OWN], f32, tag="rot",
                                        name=f"si{h}_{t2b}")
                    nc.tensor.matmul(sr_ps[:], k_t[:, t2s], q_t[:, 0, :],
                                     start=True, stop=True)
                    nc.tensor.matmul(si_ps[:], k_t[:, t2s], q_t[:, 1, :],
                                     start=True, stop=True)
                    sqr = psc.tile([128, OWN], f32, tag="sc",
                                   name=f"sqr{h}_{t2b}")
                    sqi = psc.tile([128, OWN], f32, tag="sc",
                                   name=f"sqi{h}_{t2b}")
                    nc.scalar.activation(sqr[:], sr_ps[:], AF.Square)
                    nc.scalar.activation(sqi[:], si_ps[:], AF.Square)
                    # in-place chain on sqr: m2 -> ln -> 0.5ln -> mag
                    nc.vector.tensor_tensor(sqr[:], sqr[:], sqi[:], op=ALU.add)
                    nc.scalar.activation(sqr[:], sqr[:], AF.Ln)
                    nc.scalar.activation(sqr[:], sqr[:], AF.Exp, scale=0.5)
                    et = pet.tile([128, OWN], f32r, tag="et",
                                  name=f"et{h}_{t2b}")
                    nc.scalar.activation(et[:], sqr[:], AF.Exp)
                    if debug and h == 0 and t2b == 0:
                        nc.sync.dma_start(dbg["d_exp"][:], et[:].bitcast(f32))
                    et_fifo.append((h, t2b, et, vt_pair))
                    while len(et_fifo) > LAG:
                        emit_avden(et_fifo.pop(0))
            for ent in et_fifo:
                emit_avden(ent)
            et_fifo.clear()
            es_b.close()

            # ------------- phase D: proj + residual --------------------
            r1r = poolR1.tile([128, 12, OWN], f32r, name="r1r")
            with ExitStack() as es_d:
                pwp = es_d.enter_context(tc.tile_pool(name="pwp", bufs=3))
                ps_d = es_d.enter_context(
                    tc.tile_pool(name="ps_d", bufs=4, space="PSUM"))
                for opb in range(12):
                    wp_t = pwp.tile([128, 12, 128], f32r, tag="wp",
                                    name=f"wp{opb}")
                    wps_t = pwp.tile([4, 128], f32r, tag="wps",
                                     name=f"wps{opb}")
                    nc.sync.dma_start(wp_t[:], w_proj[opb]
                                      .rearrange("b p n -> p b n"))
                    nc.sync.dma_start(wps_t[:], w_proj_s[opb])
                    pr_ps = ps_d.tile([128, OWN], f32, tag="pr",
                                      name=f"prps{opb}")
                    for kb in range(12):
                        nc.tensor.matmul(pr_ps[:], wp_t[:, kb, :],
                                         attn[:, kb, :],
                                         start=(kb == 0), stop=False)
                    nc.tensor.matmul(pr_ps[:], wps_t[:], t_stat_one[:],
                                     start=False, stop=True)
                    nc.vector.tensor_tensor(xr1[:, opb, :], pr_ps[:],
                                            xr1[:, opb, :], op=ALU.add)
                    nc.vector.tensor_copy(r1r[:, opb, :], xr1[:, opb, :])
                if debug:
                    nc.sync.dma_start(dbg["d_r1"][:], xr1[:, 0, :])
            es_attn.close()

        # ---------------- phase E: LN2 stats over own tokens --------------
        stat2 = poolR1.tile([4, OWN], f32r, name="stat2")
        rstd2_bc = poolR1.tile([128, OWN], f32, name="rstd2_bc")
        with ExitStack() as es_e:
            pe = es_e.enter_context(tc.tile_pool(name="pe_sb", bufs=1))
            pdram2 = es_e.enter_context(
                tc.tile_pool(name="pdram2", bufs=1, space="DRAM"))
            pe_ps = es_e.enter_context(
                tc.tile_pool(name="pe_ps", bufs=2, space="PSUM"))
            sq2s = []
            for kb in range(12):
                sq2 = pe.tile([128, OWN], f32r, tag="sq2", name=f"sq2_{kb}", bufs=12)
                nc.scalar.activation(sq2[:], r1r[:, kb, :], AF.Square)
                sq2s.append(sq2)
            mu2_ps = pe_ps.tile([2, OWN], f32, tag="mu2", name="mu2")
            s2_ps = pe_ps.tile([1, OWN], f32, tag="s2", name="s2")
            for kb in range(12):
                lhs = t_ones_ab[:, 0:2] if kb < 6 else t_ones_ab[:, 2:4]
                nc.tensor.matmul(mu2_ps[:], lhs, r1r[:, kb, :],
                                 start=(kb == 0), stop=(kb == 11))
                nc.tensor.matmul(s2_ps[:], t_ones_s[:], sq2s[kb][:],
                                 start=(kb == 0), stop=(kb == 11))
            mu2_sb = pe.tile([2, OWN], f32, tag="emusb", name="emusb")
            mu2_fl = pe.tile([1, 2, OWN], f32, tag="emufl", name="emufl")
            var = pe.tile([1, OWN], f32, tag="evar", name="evar")
            lnv = pe.tile([1, OWN], f32, tag="elnv", name="elnv")
            nc.vector.tensor_copy(mu2_sb[:], mu2_ps[:])
            nc.sync.dma_start(mu2_fl[:, 0, :], mu2_sb[0:1, :])
            nc.sync.dma_start(mu2_fl[:, 1, :], mu2_sb[1:2, :])
            sq_mu2 = pe.tile([1, 2, OWN], f32, tag="esqmu", name="esqmu")
            nc.vector.tensor_tensor(sq_mu2[:], mu2_fl[:], mu2_fl[:], op=ALU.mult)
            nc.vector.tensor_tensor(var[:], s2_ps[:], sq_mu2[:, 0, :],
                                    op=ALU.subtract)
            nc.vector.tensor_tensor(var[:], var[:], sq_mu2[:, 1, :],
                                    op=ALU.subtract)
            nc.scalar.activation(lnv[:], var[:], AF.Ln, bias=t_eps[:])
            nc.vector.tensor_copy(stat2[0:2, :], mu2_sb[:])
            std2_row = pe.tile([1, OWN], f32r, tag="estd", name="estd")
            nc.scalar.activation(std2_row[:], lnv[:], AF.Exp, scale=0.5)
            nc.sync.dma_start(stat2[2:3, :], std2_row[:])
            rstd2_row = pe.tile([1, OWN], f32r, tag="ers", name="ers")
            nc.scalar.activation(rstd2_row[:], lnv[:], AF.Exp, scale=-0.5)
            rstd2_dram = pdram2.tile([1, OWN], f32, name="rstd2_dram")
            nc.sync.dma_start(rstd2_dram[:], rstd2_row[:].bitcast(f32))
            bcast2 = bass.AP(tensor=rstd2_dram.tensor, offset=rstd2_dram[:].offset,
                             ap=[[0, 128]] + rstd2_dram[:].ap[1:])
            nc.sync.dma_start(rstd2_bc[:], bcast2)
            if debug:
                nc.sync.dma_start(dbg["d_stat2"][:], stat2[:].bitcast(f32))

        # ---------------- phase F: MLP per 256-token chunk ----------------
        with ExitStack() as es_f:
            pneg = es_f.enter_context(tc.tile_pool(name="pneg", bufs=1))
            r1neg = pneg.tile([128, 6, OWN], f32r, name="r1neg")
            for kb in range(6):
                nc.vector.tensor_scalar(r1neg[:, kb, :], r1r[:, 6 + kb, :],
                                        -1.0, None, op0=ALU.mult)
            ph = es_f.enter_context(tc.tile_pool(name="ph", bufs=1))
            phn = es_f.enter_context(tc.tile_pool(name="phn", bufs=1))
            pw1 = es_f.enter_context(tc.tile_pool(name="pw1", bufs=3))
            pw2 = es_f.enter_context(tc.tile_pool(name="pw2", bufs=4))
            pscf = es_f.enter_context(tc.tile_pool(name="pscf", bufs=4))
            pout = es_f.enter_context(tc.tile_pool(name="pout", bufs=2))
            ps_f = es_f.enter_context(
                tc.tile_pool(name="ps_f", bufs=6, space="PSUM"))
            for cc in range(2):
                cs = slice(cc * MC, cc * MC + MC)
                h_t = ph.tile([128, 48, MC], f32r, tag="h", name=f"h{cc}")
                hn_t = phn.tile([128, 24, MC], f32r, tag="hn", name=f"hn{cc}")
                for Cb in range(24):
                    w1r_t = pw1.tile([128, 6, 128], f32r, tag="w1r",
                                     name=f"w1r{cc}_{Cb}")
                    w1i_t = pw1.tile([128, 6, 128], f32r, tag="w1i",
                                     name=f"w1i{cc}_{Cb}")
                    w1s_t = pw1.tile([4, 256], f32r, tag="w1s",
                                     name=f"w1s{cc}_{Cb}")
                    nc.sync.dma_start(w1r_t[:], w_fc1r[Cb]
                                      .rearrange("b p n -> p b n"))
                    nc.sync.dma_start(w1i_t[:], w_fc1i[Cb]
                                      .rearrange("b p n -> p b n"))
                    nc.sync.dma_start(w1s_t[:], w_fc1_s[Cb])
                    hr_ps = ps_f.tile([128, MC], f32, tag="fps",
                                      name=f"hrps{cc}_{Cb}")
                    hi_ps = ps_f.tile([128, MC], f32, tag="fps",
                                      name=f"hips{cc}_{Cb}")
                    for kb in range(6):
                        st = (kb == 0)
                        nc.tensor.matmul(hr_ps[:], w1r_t[:, kb, :],
                                         r1r[:, kb, cs], start=st, stop=False)
                        nc.tensor.matmul(hi_ps[:], w1i_t[:, kb, :],
                                         r1r[:, kb, cs], start=st, stop=False)
                    for kb in range(6):
                        nc.tensor.matmul(hr_ps[:], w1i_t[:, kb, :],
                                         r1neg[:, kb, cs], start=False, stop=False)
                        nc.tensor.matmul(hi_ps[:], w1r_t[:, kb, :],
                                         r1r[:, 6 + kb, cs], start=False,
                                         stop=False)
                    nc.tensor.matmul(hr_ps[:], w1s_t[:, 0:128], stat2[:, cs],
                                     start=False, stop=True)
                    nc.tensor.matmul(hi_ps[:], w1s_t[:, 128:256], stat2[:, cs],
                                     start=False, stop=True)
                    gr = pscf.tile([128, MC], f32, tag="g", name=f"gr{cc}_{Cb}")
                    gi = pscf.tile([128, MC], f32, tag="g", name=f"gi{cc}_{Cb}")
                    nc.vector.tensor_tensor(gr[:], hr_ps[:], rstd2_bc[:, cs],
                                            op=ALU.mult)
                    nc.vector.tensor_tensor(gi[:], hi_ps[:], rstd2_bc[:, cs],
                                            op=ALU.mult)
                    nc.scalar.activation(h_t[:, Cb, :], gr[:], AF.Gelu)
                    nc.scalar.activation(h_t[:, 24 + Cb, :], gi[:], AF.Gelu)
                    nc.vector.tensor_scalar(hn_t[:, Cb, :], h_t[:, 24 + Cb, :],
                                            -1.0, None, op0=ALU.mult)
                if debug and cc == 0:
                    nc.sync.dma_start(dbg["d_h"][:], h_t[:, 0, :].bitcast(f32))
                for j in range(6):
                    w2r_a = pw2.tile([128, 12, 128], f32r, tag="w2",
                                     name=f"w2ra{cc}_{j}")
                    w2r_b = pw2.tile([128, 12, 128], f32r, tag="w2",
                                     name=f"w2rb{cc}_{j}")
                    w2i_a = pw2.tile([128, 12, 128], f32r, tag="w2",
                                     name=f"w2ia{cc}_{j}")
                    w2i_b = pw2.tile([128, 12, 128], f32r, tag="w2",
                                     name=f"w2ib{cc}_{j}")
                    w2s_t = pw2.tile([4, 256], f32r, tag="w2s",
                                     name=f"w2s{cc}_{j}")
                    nc.sync.dma_start(w2r_a[:], w_fc2r[j, 0:12]
                                      .rearrange("b p n -> p b n"))
                    nc.sync.dma_start(w2r_b[:], w_fc2r[j, 12:24]
                                      .rearrange("b p n -> p b n"))
                    nc.sync.dma_start(w2i_a[:], w_fc2i[j, 0:12]
                                      .rearrange("b p n -> p b n"))
                    nc.sync.dma_start(w2i_b[:], w_fc2i[j, 12:24]
                                      .rearrange("b p n -> p b n"))
                    nc.sync.dma_start(w2s_t[:], w_fc2_s[j])
                    or_ps = ps_f.tile([128, MC], f32, tag="fps",
                                      name=f"orps{cc}_{j}")
                    oi_ps = ps_f.tile([128, MC], f32, tag="fps",
                                      name=f"oips{cc}_{j}")
                    for kb in range(24):
                        w2r = w2r_a[:, kb, :] if kb < 12 else w2r_b[:, kb - 12, :]
                        w2i = w2i_a[:, kb, :] if kb < 12 else w2i_b[:, kb - 12, :]
                        st = (kb == 0)
                        nc.tensor.matmul(or_ps[:], w2r, h_t[:, kb, :],
                                         start=st, stop=False)
                        nc.tensor.matmul(oi_ps[:], w2i, h_t[:, kb, :],
                                         start=st, stop=False)
                    for kb in range(24):
                        w2r = w2r_a[:, kb, :] if kb < 12 else w2r_b[:, kb - 12, :]
                        w2i = w2i_a[:, kb, :] if kb < 12 else w2i_b[:, kb - 12, :]
                        nc.tensor.matmul(or_ps[:], w2i, hn_t[:, kb, :],
                                         start=False, stop=False)
                        nc.tensor.matmul(oi_ps[:], w2r,
                                         h_t[:, 24 + kb, :],
                                         start=False, stop=False)
                    nc.tensor.matmul(or_ps[:], w2s_t[:, 0:128],
                                     t_stat_one[:, cs], start=False, stop=True)
                    nc.tensor.matmul(oi_ps[:], w2s_t[:, 128:256],
                                     t_stat_one[:, cs], start=False, stop=True)
                    o_r = pout.tile([128, MC], f32, tag="o", name=f"or{cc}_{j}")
                    o_i = pout.tile([128, MC], f32, tag="o", name=f"oi{cc}_{j}")
                    nc.vector.tensor_tensor(o_r[:], or_ps[:], xr1[:, j, cs],
                                            op=ALU.add)
                    nc.vector.tensor_tensor(o_i[:], oi_ps[:], xr1[:, 6 + j, cs],
                                            op=ALU.add)
                    nc.sync.dma_start(out_fm[j, :, cs], o_r[:])
                    nc.sync.dma_start(out_fm[6 + j, :, cs], o_i[:])
    nc.compile()
    return nc




# --------------------------------------------------------------------------
# host side
# --------------------------------------------------------------------------

def _cx(a):
    return a[..., 0].astype(np.float64) + 1j * a[..., 1].astype(np.float64)


def _kcols(Wp, wsum, wb, plane, scale=1.0):
    """K-profile [1539, m] for output features with complex weight rows Wp
    [m, 768], LN fold sums wsum [m], bias-column wb [m]. K rows: xr(768),
    xi(768), mu_r, mu_i, std."""
    m = Wp.shape[0]
    out = np.zeros((1539, m), np.float64)
    if plane == "r":
        out[0:768] = Wp.real.T
        out[768:1536] = -Wp.imag.T
        out[1536] = -wsum.real
        out[1537] = wsum.imag
        out[1538] = wb.real
    else:
        out[0:768] = Wp.imag.T
        out[768:1536] = Wp.real.T
        out[1536] = -wsum.imag
        out[1537] = -wsum.real
        out[1538] = wb.imag
    return out * scale


def _prep_weights(inputs):
    n1 = _cx(inputs["n1_w"]); b1 = _cx(inputs["n1_b"])
    n2 = _cx(inputs["n2_w"]); b2 = _cx(inputs["n2_b"])
    Wqkv = _cx(inputs["qkv_w"])          # [2304, 768]
    Wp = _cx(inputs["proj_w"])           # [768, 768]
    bp = _cx(inputs["proj_b"])           # [768]
    W1 = _cx(inputs["fc1_w"])            # [3072, 768]
    bf1 = _cx(inputs["fc1_b"])           # [3072]
    W2 = _cx(inputs["fc2_w"])            # [768, 3072]
    bf2 = _cx(inputs["fc2_b"])           # [768]

    d = {}
    # ---- qkv (LN1-folded) ----
    Wq, Wk, Wv = Wqkv[0:768], Wqkv[768:1536], Wqkv[1536:2304]

    def fold1(W):
        Wf = W * n1[None, :]
        return Wf, Wf.sum(1), W @ b1

    w_qkv = np.zeros((H, 12, 128, 384), np.float32)
    w_qkv_s = np.zeros((H, 4, 384), np.float32)
    for h in range(H):
        rows = slice(h * DH, (h + 1) * DH)
        Qf, Qs, Qb = fold1(Wq[rows])
        Kf, Ks, Kb_ = fold1(Wk[rows])
        q1 = np.hstack([_kcols(Qf, Qs, Qb, "r", SCALE),
                        _kcols(Qf, Qs, Qb, "i", -SCALE)])
        q3 = np.hstack([_kcols(Qf, Qs, Qb, "i", SCALE),
                        _kcols(Qf, Qs, Qb, "r", SCALE)])
        kk = np.hstack([_kcols(Kf, Ks, Kb_, "r"), _kcols(Kf, Ks, Kb_, "i")])
        blk = np.hstack([q1, q3, kk]).astype(np.float32)       # [1539, 384]
        w_qkv[h] = blk[0:1536].reshape(12, 128, 384)
        w_qkv_s[h, 0:3] = blk[1536:1539]
    d["w_qkv"] = round_fp32r(w_qkv)
    d["w_qkv_s"] = round_fp32r(w_qkv_s)

    # ---- v (LN1-folded), rhs layout [K, 1536]; cols: pair*256+slot*128+plane*64+dh
    wv_full = np.zeros((1539, 1536), np.float64)
    for h in range(H):
        rows = slice(h * DH, (h + 1) * DH)
        Vf, Vs, Vb = fold1(Wv[rows])
        base = h * 128
        wv_full[:, base:base + 64] = _kcols(Vf, Vs, Vb, "r")
        wv_full[:, base + 64:base + 128] = _kcols(Vf, Vs, Vb, "i")
    d["w_v"] = round_fp32r(wv_full[0:1536].reshape(12, 128, 1536))
    wvs = np.zeros((4, 1536), np.float32)
    wvs[0:3] = wv_full[1536:1539]
    d["w_v_s"] = round_fp32r(wvs)

    # ---- proj (plain + bias); K rows = attn features: per head [a_r(64); a_i(64)]
    w_proj = np.zeros((12, 12, 128, 128), np.float32)
    w_proj_s = np.zeros((12, 4, 128), np.float32)
    for opb in range(12):
        plane = "r" if opb < 6 else "i"
        orow = slice((opb % 6) * 128, (opb % 6) * 128 + 128)
        Wpo = Wp[orow]                               # [128, 768] complex
        prof = np.zeros((1536, 128), np.float64)
        for hh in range(H):
            cols = slice(hh * DH, (hh + 1) * DH)
            if plane == "r":
                prof[hh * 128:hh * 128 + 64] = Wpo.real[:, cols].T
                prof[hh * 128 + 64:hh * 128 + 128] = -Wpo.imag[:, cols].T
            else:
                prof[hh * 128:hh * 128 + 64] = Wpo.imag[:, cols].T
                prof[hh * 128 + 64:hh * 128 + 128] = Wpo.real[:, cols].T
        w_proj[opb] = prof.reshape(12, 128, 128)
        w_proj_s[opb, 0] = (bp.real if plane == "r" else bp.imag)[orow]
    d["w_proj"] = round_fp32r(w_proj)
    d["w_proj_s"] = round_fp32r(w_proj_s)

    # ---- fc1 (LN2-folded, shared-tile form) ----
    W1f = W1 * n2[None, :]
    W1s = W1f.sum(1)
    W1b = W1 @ b2 + bf1
    w_fc1r = np.zeros((24, 6, 128, 128), np.float32)
    w_fc1i = np.zeros((24, 6, 128, 128), np.float32)
    w_fc1_s = np.zeros((24, 4, 256), np.float32)
    for Cb in range(24):
        orow = slice(Cb * 128, (Cb + 1) * 128)
        for kb in range(6):
            icol = slice(kb * 128, (kb + 1) * 128)
            w_fc1r[Cb, kb] = W1f.real[orow, icol].T
            w_fc1i[Cb, kb] = W1f.imag[orow, icol].T
        w_fc1_s[Cb, 0, 0:128] = -W1s.real[orow]
        w_fc1_s[Cb, 1, 0:128] = W1s.imag[orow]
        w_fc1_s[Cb, 2, 0:128] = W1b.real[orow]
        w_fc1_s[Cb, 0, 128:256] = -W1s.imag[orow]
        w_fc1_s[Cb, 1, 128:256] = -W1s.real[orow]
        w_fc1_s[Cb, 2, 128:256] = W1b.imag[orow]
    d["w_fc1r"] = round_fp32r(w_fc1r)
    d["w_fc1i"] = round_fp32r(w_fc1i)
    d["w_fc1_s"] = round_fp32r(w_fc1_s)

    # ---- fc2 (plain + bias) ----
    w_fc2r = np.zeros((6, 24, 128, 128), np.float32)
    w_fc2i = np.zeros((6, 24, 128, 128), np.float32)
    w_fc2_s = np.zeros((6, 4, 256), np.float32)
    for j in range(6):
        orow = slice(j * 128, (j + 1) * 128)
        for kb in range(24):
            icol = slice(kb * 128, (kb + 1) * 128)
            w_fc2r[j, kb] = W2.real[orow, icol].T
            w_fc2i[j, kb] = W2.imag[orow, icol].T
        w_fc2_s[j, 0, 0:128] = bf2.real[orow]
        w_fc2_s[j, 0, 128:256] = bf2.imag[orow]
    d["w_fc2r"] = round_fp32r(w_fc2r)
    d["w_fc2i"] = round_fp32r(w_fc2i)
    d["w_fc2_s"] = round_fp32r(w_fc2_s)

    # ---- consts ----
    d["ones_col"] = np.ones((128, 1), np.float32)
    d["ones_row"] = np.ones((1, 128), np.float32)
    oab = np.zeros((128, 4), np.float32)
    oab[:, 0] = 1.0 / C
    oab[:, 3] = 1.0 / C
    d["ones_ab"] = round_fp32r(oab)
    d["ones_s"] = round_fp32r(np.full((128, 1), 1.0 / C, np.float32))
    so = np.zeros((4, OWN), np.float32)
    so[0] = 1.0
    d["stat_one"] = so
    return d


_NC_CACHE = {}


def kernel(**inputs):
    debug = bool(inputs.pop("_debug", False))
    if debug not in _NC_CACHE:
        _NC_CACHE[debug] = build_nc(debug=debug)
    nc = _NC_CACHE[debug]

    shared = _prep_weights(inputs)
    x = np.asarray(inputs["x"], np.float32)          # [B, N, C, 2]

    in_maps = []
    for c in range(NCORES):
        b, half = divmod(c, 2)
        xr_ = x[b, :, :, 0].T                        # [768, 1024]
        xi_ = x[b, :, :, 1].T
        stack = np.concatenate([xr_, xi_], 0)        # [1536, 1024]
        if half == 1:
            stack = np.concatenate([stack[:, OWN:], stack[:, :OWN]], 1)
        m = dict(shared)
        m["x_r"] = round_fp32r(stack).reshape(12, 128, N)
        m["x_own"] = np.ascontiguousarray(stack[:, 0:OWN]).reshape(12, 128, OWN)
        in_maps.append(m)

    res = run_bass_kernel_spmd(nc, in_maps, list(range(NCORES)))
    out = np.empty((B, N, C, 2), np.float32)
    for c in range(NCORES):
        b, half = divmod(c, 2)
        o = res.results[c]["out_fm"]                 # [12, 128, OWN]
        sl = slice(half * OWN, half * OWN + OWN)
        out[b, sl, :, 0] = o[0:6].reshape(768, OWN).T
        out[b, sl, :, 1] = o[6:12].reshape(768, OWN).T
    if debug:
        return out, res
    return out



# revision 6
# speedup vs baseline: 1.3555x; 1.3555x over previous
"""Complex transformer block (LN->attn->LN->MLP, complex arithmetic) on 8 TRN2 cores.

Sharding: core c handles (batch b = c//2, sequence half = c%2). No collectives:
each core computes K/V over the full 1024-token sequence of its batch (the only
duplicated work) and queries/MLP over its own 512 tokens.

Layout: activations are feature-major [feature partition-blocks, tokens].
Complex tensors are realified as separate real/imag feature planes. LayerNorm
is fused into the following matmul: per-token stats (mu_r, mu_i, std) are
appended as 3 extra contraction rows with matching weight columns, and the
per-token rstd is applied by the PSUM-eviction multiply. Attention scores are
computed transposed ([t2, t1]) so softmax sums reduce via ones-matmuls, and V
is produced pre-transposed by swapping matmul operands.

All weights are stored bf16 host-side in the exact SBUF layout (partition-major)
so every weight DMA is a single fully-contiguous transfer, and bf16 stationaries
get the fast-weight-load (FWL) path on the PE. Activations stay float32r except
k/vt/et (matmul stationaries or moving operands where bf16 keeps full PE rate).
"""
import sys
sys.path.insert(0, "/opt/trn_rl_repo")

from contextlib import ExitStack

import ml_dtypes
import numpy as np

import concourse.bacc as bacc
import concourse.bass as bass
import concourse.mybir as mybir
import concourse.tile as tile
from concourse.bass_utils import run_bass_kernel_spmd

# Prefer the table set that covers the whole softmax chain (square+ln+exp)
# so the greedy act-table-load pass doesn't thrash sets on every block.
_orig_get_tables = bacc.get_activation_tables


def _reordered_tables(arch):
    # Keep canonical order/indices (walrus resolves act_func_set_id by
    # position) but blank every set except the two we want, so the greedy
    # table-load pass can't thrash between sets per softmax block.
    t = _orig_get_tables(arch)
    keep = {"natural_log_exp_and_others", "gelu_and_others"}
    return {k: (v if k in keep else set()) for k, v in t.items()}


bacc.get_activation_tables = _reordered_tables

dt = mybir.dt
AF = mybir.ActivationFunctionType
ALU = mybir.AluOpType
BF16 = ml_dtypes.bfloat16

B, N, C, H, DH, HID = 4, 1024, 768, 12, 64, 3072
NCORES = 8
OWN = 512          # tokens per core
KB = C // 128      # 6 feature pblocks per plane
SCALE = DH ** -0.5
EPS = 1e-5


def round_fp32r(x):
    b = np.ascontiguousarray(x, dtype=np.float32).view(np.uint32)
    lsb = (b >> np.uint32(12)) & np.uint32(1)
    return ((b + np.uint32(0x7FF) + lsb) & np.uint32(0xFFFFF000)).view(np.float32)


# --------------------------------------------------------------------------
# device program
# --------------------------------------------------------------------------

def build_nc(debug=False):
    nc = bacc.Bacc(trn_type="TRN2", target_bir_lowering=False)
    f32 = dt.float32
    f32r = dt.float32r
    bf16 = dt.bfloat16

    # ---- DRAM I/O ----
    x_r = nc.dram_tensor("x_r", [128, 12, N], bf16, kind="ExternalInput")
    x_own = nc.dram_tensor("x_own", [128, 12, OWN], f32, kind="ExternalInput")
    w_qkv = nc.dram_tensor("w_qkv", [H, 128, 12, 384], bf16, kind="ExternalInput")
    w_qkv_s = nc.dram_tensor("w_qkv_s", [H, 4, 384], f32r, kind="ExternalInput")
    w_v = nc.dram_tensor("w_v", [6, 128, 12, 256], bf16, kind="ExternalInput")
    w_v_s = nc.dram_tensor("w_v_s", [4, 1536], f32r, kind="ExternalInput")
    w_proj = nc.dram_tensor("w_proj", [12, 128, 12, 128], bf16, kind="ExternalInput")
    w_proj_s = nc.dram_tensor("w_proj_s", [12, 4, 128], f32r, kind="ExternalInput")
    w_fc1r = nc.dram_tensor("w_fc1r", [24, 128, 6, 128], bf16, kind="ExternalInput")
    w_fc1i = nc.dram_tensor("w_fc1i", [24, 128, 6, 128], bf16, kind="ExternalInput")
    w_fc1_s = nc.dram_tensor("w_fc1_s", [24, 4, 256], f32r, kind="ExternalInput")
    w_fc2r = nc.dram_tensor("w_fc2r", [6, 128, 24, 128], bf16, kind="ExternalInput")
    w_fc2i = nc.dram_tensor("w_fc2i", [6, 128, 24, 128], bf16, kind="ExternalInput")
    w_fc2_s = nc.dram_tensor("w_fc2_s", [6, 4, 256], f32r, kind="ExternalInput")
    ones_col = nc.dram_tensor("ones_col", [128, 1], bf16, kind="ExternalInput")
    ones_ab = nc.dram_tensor("ones_ab", [128, 4], bf16, kind="ExternalInput")
    # ones_ab cols: [1/C, 0] (A: xr-plane mu), [0, 1/C] (B: xi-plane mu)
    ones_s = nc.dram_tensor("ones_s", [128, 1], bf16, kind="ExternalInput")
    stat_one = nc.dram_tensor("stat_one", [4, OWN], f32r, kind="ExternalInput")
    ident8 = nc.dram_tensor("ident8", [8, 8], f32r, kind="ExternalInput")

    out_fm = nc.dram_tensor("out_fm", [12, 128, OWN], f32, kind="ExternalOutput")
    dbg = {}
    if debug:
        for nm, shp in [
            ("d_stat1", [4, N]), ("d_q1", [128, OWN]), ("d_k", [128, N]),
            ("d_vt", [128, 8, 256]), ("d_exp", [128, OWN]), ("d_den", [1, OWN]),
            ("d_attn", [128, OWN]), ("d_r1", [128, OWN]), ("d_stat2", [4, OWN]),
            ("d_h", [128, OWN]), ("d_S", [1, 512]), ("d_var", [1, 512]),
            ("d_mufl", [1, 2, 512]), ("d_sq0", [128, 512]),
            ("d_rstdT", [128, 8]),
        ]:
            dbg[nm] = nc.dram_tensor(nm, shp, f32, kind="ExternalOutput")

    with tile.TileContext(nc) as tc, ExitStack() as top:
        consts = top.enter_context(tc.tile_pool(name="consts", bufs=1))
        t_ones_col = consts.tile([128, 1], bf16)
        t_ones_ab = consts.tile([128, 4], bf16)
        t_ones_s = consts.tile([128, 1], bf16)
        t_stat_one = consts.tile([4, OWN], f32r)
        t_id8 = consts.tile([8, 8], f32r)
        t_eps = consts.tile([1, 1], f32)
        nc.sync.dma_start(t_ones_col[:], ones_col[:])
        nc.sync.dma_start(t_ones_ab[:], ones_ab[:])
        nc.sync.dma_start(t_ones_s[:], ones_s[:])
        nc.sync.dma_start(t_stat_one[:], stat_one[:])
        nc.sync.dma_start(t_id8[:], ident8[:])
        nc.vector.memset(t_eps[:], EPS)

        poolR1 = top.enter_context(tc.tile_pool(name="poolR1", bufs=1))
        xr1 = poolR1.tile([128, 12, OWN], f32, name="xr1")
        nc.sync.dma_start(xr1[:], x_own[:])

        with ExitStack() as es_x:
            poolX = es_x.enter_context(tc.tile_pool(name="poolX", bufs=1))
            xrb = poolX.tile([128, 12, N], bf16, name="xrb")
            pdram = es_x.enter_context(
                tc.tile_pool(name="pdram", bufs=1, space="DRAM"))
            rstd_dram = pdram.tile([1, N], f32, name="rstd_dram")
            stat1 = poolX.tile([4, N], f32r, name="stat1")
            rstd_bc1 = poolX.tile([128, N], f32, name="rstd_bc1")
            rstdT = poolX.tile([128, 8], f32, name="rstdT")
            nc.sync.dma_start(xrb[:], x_r[:])

            # ---------------- phase A: LN1 stats over full sequence --------
            with ExitStack() as es_a:
                pa = es_a.enter_context(tc.tile_pool(name="pa_sb", bufs=3))
                pa_ps = es_a.enter_context(
                    tc.tile_pool(name="pa_ps", bufs=2, space="PSUM"))
                pa_sc = es_a.enter_context(tc.tile_pool(name="pa_sc", bufs=1))
                mu_pss = [pa_ps.tile([2, 512], f32, tag=f"mu{ch}",
                                     name=f"mu{ch}", bufs=1) for ch in range(2)]
                s_pss = [pa_ps.tile([1, 512], f32, tag=f"s{ch}",
                                    name=f"s{ch}", bufs=1) for ch in range(2)]
                for kb in range(12):
                    sq = pa.tile([128, N], bf16, tag="sq", name=f"sq{kb}")
                    nc.scalar.activation(sq[:], xrb[:, kb, :], AF.Square)
                    lhs = t_ones_ab[:, 0:2] if kb < 6 else t_ones_ab[:, 2:4]
                    for ch in range(2):
                        sl = slice(ch * 512, ch * 512 + 512)
                        nc.tensor.matmul(mu_pss[ch][:], lhs, xrb[:, kb, sl],
                                         start=(kb == 0), stop=(kb == 11))
                        nc.tensor.matmul(s_pss[ch][:], t_ones_s[:], sq[:, sl],
                                         start=(kb == 0), stop=(kb == 11))
                for ch in range(2):
                    sl = slice(ch * 512, ch * 512 + 512)
                    mu_ps = mu_pss[ch]
                    s_ps = s_pss[ch]
                    # var = S - mu_r^2 - mu_i^2 ; std = exp(.5 ln(var+eps))
                    mu_sb = pa_sc.tile([2, 512], f32, tag="musb", name=f"musb{ch}")
                    mu_fl = pa_sc.tile([1, 2, 512], f32, tag="mufl", name=f"mufl{ch}")
                    var = pa_sc.tile([1, 512], f32, tag="var", name=f"var{ch}")
                    lnv = pa_sc.tile([1, 512], f32, tag="lnv", name=f"lnv{ch}")
                    s_c = pa_sc.tile([1, 512], f32, tag="sc_", name=f"sc_{ch}")
                    nc.vector.tensor_scalar(mu_sb[:], mu_ps[:], 1.0 / C, None,
                                            op0=ALU.mult)
                    nc.vector.tensor_scalar(s_c[:], s_ps[:], 1.0 / C, None,
                                            op0=ALU.mult)
                    nc.sync.dma_start(mu_fl[:, 0, :], mu_sb[0:1, :])
                    nc.sync.dma_start(mu_fl[:, 1, :], mu_sb[1:2, :])
                    sq_mu = pa_sc.tile([1, 2, 512], f32, tag="sqmu", name=f"sqmu{ch}")
                    nc.vector.tensor_tensor(sq_mu[:], mu_fl[:], mu_fl[:],
                                            op=ALU.mult)
                    nc.vector.tensor_tensor(var[:], s_c[:], sq_mu[:, 0, :],
                                            op=ALU.subtract)
                    nc.vector.tensor_tensor(var[:], var[:], sq_mu[:, 1, :],
                                            op=ALU.subtract)
                    nc.scalar.activation(lnv[:], var[:], AF.Ln, bias=t_eps[:])
                    if debug and ch == 0:
                        nc.sync.dma_start(dbg["d_var"][:], var[:])
                        nc.sync.dma_start(dbg["d_mufl"][:], mu_fl[:])
                        s_sb_dbg = pa_sc.tile([1, 512], f32, tag="sdbg",
                                              name="sdbg")
                        nc.vector.tensor_copy(s_sb_dbg[:], s_ps[:])
                        nc.sync.dma_start(dbg["d_S"][:], s_sb_dbg[:])
                    # stats rows: 0=mu_r 1=mu_i 2=std
                    nc.vector.tensor_copy(stat1[0:2, sl], mu_sb[:])
                    std_row = pa_sc.tile([1, 512], f32r, tag="stdr", name=f"stdr{ch}")
                    nc.scalar.activation(std_row[:], lnv[:], AF.Exp, scale=0.5)
                    nc.sync.dma_start(stat1[2:3, sl], std_row[:])
                    rstd_row = pa_sc.tile([1, 512], f32r, tag="rst", name=f"rst{ch}")
                    nc.scalar.activation(rstd_row[:], lnv[:], AF.Exp, scale=-0.5)
                    nc.sync.dma_start(rstd_dram[:, sl], rstd_row[:].bitcast(f32))
                    bcast = bass.AP(tensor=rstd_dram.tensor,
                                    offset=rstd_dram[:, sl].offset,
                                    ap=[[0, 128]] + rstd_dram[:, sl].ap[1:])
                    nc.sync.dma_start(rstd_bc1[:, sl], bcast)
                # rstd transposed: rstdT[p, t2b] = rstd[t2b*128 + p]
                rstd8 = pa_sc.tile([8, 128], f32, tag="r8", name="rstd8")
                nc.sync.dma_start(
                    rstd8[:], rstd_dram[:].rearrange("o (a b) -> (o a) b", a=8))
                rstdT_ps = pa_ps.tile([128, 8], f32, tag="rtps", name="rtps")
                nc.tensor.transpose(rstdT_ps[:], rstd8[:], t_id8[:].bitcast(f32))
                nc.vector.tensor_copy(rstdT[:], rstdT_ps[:])
                if debug:
                    nc.sync.dma_start(dbg["d_stat1"][:],
                                      stat1[:].bitcast(f32))
                    nc.sync.dma_start(dbg["d_rstdT"][:], rstdT[:])

            # ---------------- phase BC: qkv + attention per head ----------
            es_attn = ExitStack()
            attnp = es_attn.enter_context(tc.tile_pool(name="attnp", bufs=1))
            attn = attnp.tile([128, 12, OWN], bf16, name="attn")
            es_b = ExitStack()
            pq = es_b.enter_context(tc.tile_pool(name="pq", bufs=1))
            pk = es_b.enter_context(tc.tile_pool(name="pk", bufs=1))
            pvt = es_b.enter_context(tc.tile_pool(name="pvt", bufs=2))
            pwv = es_b.enter_context(tc.tile_pool(name="pwv", bufs=1))
            pwq = es_b.enter_context(tc.tile_pool(name="pwq", bufs=2))
            pet = es_b.enter_context(tc.tile_pool(name="pet", bufs=6))
            psc = es_b.enter_context(tc.tile_pool(name="psc", bufs=6))
            prd = es_b.enter_context(tc.tile_pool(name="prd", bufs=2))
            ps_rot = es_b.enter_context(
                tc.tile_pool(name="ps_rot", bufs=6, space="PSUM"))
            ps_acc = es_b.enter_context(
                tc.tile_pool(name="ps_acc", bufs=2, space="PSUM"))
            pdram_rd = es_b.enter_context(
                tc.tile_pool(name="pdram_rd", bufs=2, space="DRAM"))
            vt_pair = None
            et_fifo = []
            acc_ps = {}
            LAG = 4

            def emit_avden(ent):
                h2, t2b2, et2, vt2 = ent
                slot2 = h2 % 2
                if t2b2 == 0:
                    acc_ps[h2] = (
                        ps_acc.tile([128, OWN], f32, tag="av", name=f"av{h2}",
                                    bufs=1),
                        ps_acc.tile([1, OWN], f32, tag="den", name=f"den{h2}",
                                    bufs=1),
                    )
                av2, den2 = acc_ps[h2]
                nc.tensor.matmul(den2[:], t_ones_col[:], et2[:],
                                 start=(t2b2 == 0), stop=(t2b2 == 7))
                dsl2 = slice(slot2 * 128, slot2 * 128 + 128)
                nc.tensor.matmul(av2[:], vt2[:, t2b2, dsl2], et2[:],
                                 start=(t2b2 == 0), stop=(t2b2 == 7))
                if t2b2 == 7:
                    den_sb = prd.tile([1, OWN], f32, tag="den_sb",
                                      name=f"dsb{h2}", bufs=1)
                    rd = prd.tile([1, OWN], f32, tag="rd", name=f"rd{h2}",
                                  bufs=1)
                    nc.vector.tensor_copy(den_sb[:], den2[:])
                    nc.vector.reciprocal(rd[:], den_sb[:])
                    rd_dram = pdram_rd.tile([1, OWN], f32, tag="rdd",
                                            name=f"rdd{h2}")
                    nc.sync.dma_start(rd_dram[:], rd[:])
                    rd_bc = prd.tile([128, OWN], f32, tag="rd_bc",
                                     name=f"rdbc{h2}", bufs=1)
                    rd_bcast_ap = bass.AP(tensor=rd_dram.tensor,
                                          offset=rd_dram[:].offset,
                                          ap=[[0, 128]] + rd_dram[:].ap[1:])
                    nc.sync.dma_start(rd_bc[:], rd_bcast_ap)
                    nc.vector.tensor_tensor(attn[:, h2, :], av2[:], rd_bc[:],
                                            op=ALU.mult)
                    del acc_ps[h2]
                    if debug and h2 == 0:
                        nc.sync.dma_start(dbg["d_den"][:], den_sb[:])
                        nc.sync.dma_start(dbg["d_attn"][:],
                                          attn[:, 0, :].bitcast(f32))

            for h in range(H):
                pair, slot = divmod(h, 2)
                # qkv for head h: q1=[q_r;-q_i], q3=[q_i;q_r], k=[k_r;k_i]
                q_t = pq.tile([128, 2, OWN], bf16, tag="q", name=f"q{h}")
                k_t = pk.tile([128, N], bf16, tag="k", name=f"k{h}")
                wqkv_t = pwq.tile([128, 12, 384], bf16, tag="wqkv",
                                  name=f"wqkv{h}")
                wqs_t = pwq.tile([4, 384], f32r, tag="wqs", name=f"wqs{h}")
                nc.sync.dma_start(wqkv_t[:], w_qkv[h])
                nc.sync.dma_start(wqs_t[:], w_qkv_s[h])
                q1_ps = ps_rot.tile([128, OWN], f32, tag="rot", name=f"q1ps{h}")
                q3_ps = ps_rot.tile([128, OWN], f32, tag="rot", name=f"q3ps{h}")
                for kb in range(12):
                    st = (kb == 0)
                    nc.tensor.matmul(q1_ps[:], wqkv_t[:, kb, 0:128],
                                     xrb[:, kb, 0:OWN], start=st, stop=False)
                    nc.tensor.matmul(q3_ps[:], wqkv_t[:, kb, 128:256],
                                     xrb[:, kb, 0:OWN], start=st, stop=False)
                nc.tensor.matmul(q1_ps[:], wqs_t[:, 0:128], stat1[:, 0:OWN],
                                 start=False, stop=True)
                nc.tensor.matmul(q3_ps[:], wqs_t[:, 128:256], stat1[:, 0:OWN],
                                 start=False, stop=True)
                nc.vector.tensor_tensor(q_t[:, 0, :], q1_ps[:],
                                        rstd_bc1[:, 0:OWN], op=ALU.mult)
                nc.vector.tensor_tensor(q_t[:, 1, :], q3_ps[:],
                                        rstd_bc1[:, 0:OWN], op=ALU.mult)
                k0_ps = ps_rot.tile([128, 512], f32, tag="rot", name=f"k0ps{h}")
                k1_ps = ps_rot.tile([128, 512], f32, tag="rot", name=f"k1ps{h}")
                for kb in range(12):
                    st = (kb == 0)
                    nc.tensor.matmul(k0_ps[:], wqkv_t[:, kb, 256:384],
                                     xrb[:, kb, 0:512], start=st, stop=False)
                    nc.tensor.matmul(k1_ps[:], wqkv_t[:, kb, 256:384],
                                     xrb[:, kb, 512:N], start=st, stop=False)
                nc.tensor.matmul(k0_ps[:], wqs_t[:, 256:384], stat1[:, 0:512],
                                 start=False, stop=True)
                nc.tensor.matmul(k1_ps[:], wqs_t[:, 256:384], stat1[:, 512:N],
                                 start=False, stop=True)
                nc.vector.tensor_tensor(k_t[:, 0:512], k0_ps[:],
                                        rstd_bc1[:, 0:512], op=ALU.mult)
                nc.vector.tensor_tensor(k_t[:, 512:N], k1_ps[:],
                                        rstd_bc1[:, 512:N], op=ALU.mult)
                if debug and h == 0:
                    nc.sync.dma_start(dbg["d_q1"][:],
                                      q_t[:, 0, :].bitcast(f32))
                    k_dbg = prd.tile([128, N], f32, tag="kdbg", name="kdbg")
                    nc.vector.tensor_copy(k_dbg[:], k_t[:])
                    nc.sync.dma_start(dbg["d_k"][:], k_dbg[:])
                if slot == 0:
                    # V^T for this head pair: [t2, d] via swapped operands
                    wv_t = pwv.tile([128, 12, 256], bf16, tag="wv",
                                    name=f"wv{pair}")
                    wv_s = pwv.tile([4, 256], f32r, tag="wvs",
                                    name=f"wvs{pair}")
                    csl = slice(pair * 256, pair * 256 + 256)
                    nc.sync.dma_start(wv_t[:], w_v[pair])
                    nc.sync.dma_start(wv_s[:], w_v_s[:, csl])
                    vt_pair = pvt.tile([128, 8, 256], bf16, tag="vt",
                                       name=f"vt{pair}")
                    for t2b in range(8):
                        t2s = slice(t2b * 128, t2b * 128 + 128)
                        vt_ps = ps_rot.tile([128, 256], f32, tag="rot",
                                            name=f"vtps{pair}_{t2b}")
                        for kb in range(12):
                            nc.tensor.matmul(vt_ps[:], xrb[:, kb, t2s],
                                             wv_t[:, kb, :],
                                             start=(kb == 0), stop=False)
                        nc.tensor.matmul(vt_ps[:], stat1[:, t2s], wv_s[:],
                                         start=False, stop=True)
                        nc.vector.tensor_scalar(
                            vt_pair[:, t2b, :], vt_ps[:],
                            rstdT[:, t2b:t2b + 1], None, op0=ALU.mult)
                    if debug and pair == 0:
                        vt_dbg = prd.tile([128, 8, 256], f32, tag="vtdbg",
                                          name="vtdbg")
                        nc.vector.tensor_copy(vt_dbg[:], vt_pair[:])
                        nc.sync.dma_start(dbg["d_vt"][:], vt_dbg[:])
                # scores + exp chain; den/av matmuls lag by LAG blocks
                for t2b in range(8):
                    t2s = slice(t2b * 128, t2b * 128 + 128)
                    sr_ps = ps_rot.tile([128, OWN], f32, tag="rot",
                                        name=f"sr{h}_{t2b}")
                    si_ps = ps_rot.tile([128, OWN], f32, tag="rot",
                                        name=f"si{h}_{t2b}")
                    nc.tensor.matmul(sr_ps[:], k_t[:, t2s], q_t[:, 0, :],
                                     start=True, stop=True)
                    nc.tensor.matmul(si_ps[:], k_t[:, t2s], q_t[:, 1, :],
                                     start=True, stop=True)
                    sqr = psc.tile([128, OWN], f32, tag="sc",
                                   name=f"sqr{h}_{t2b}")
                    sqi = psc.tile([128, OWN], f32, tag="sc",
                                   name=f"sqi{h}_{t2b}")
                    nc.scalar.activation(sqr[:], sr_ps[:], AF.Square)
                    nc.scalar.activation(sqi[:], si_ps[:], AF.Square)
                    # in-place chain on sqr: m2 -> ln -> 0.5ln -> mag
                    nc.vector.tensor_tensor(sqr[:], sqr[:], sqi[:], op=ALU.add)
                    nc.scalar.activation(sqr[:], sqr[:], AF.Ln)
                    nc.scalar.activation(sqr[:], sqr[:], AF.Exp, scale=0.5)
                    et = pet.tile([128, OWN], bf16, tag="et",
                                  name=f"et{h}_{t2b}")
                    nc.scalar.activation(et[:], sqr[:], AF.Exp)
                    if debug and h == 0 and t2b == 0:
                        et_dbg = prd.tile([128, OWN], f32, tag="etdbg",
                                          name="etdbg")
                        nc.vector.tensor_copy(et_dbg[:], et[:])
                        nc.sync.dma_start(dbg["d_exp"][:], et_dbg[:])
                    et_fifo.append((h, t2b, et, vt_pair))
                    while len(et_fifo) > LAG:
                        emit_avden(et_fifo.pop(0))
            for ent in et_fifo:
                emit_avden(ent)
            et_fifo.clear()
            es_b.close()

            # ------------- phase D: proj + residual --------------------
            r1r = poolR1.tile([128, 12, OWN], bf16, name="r1r")
            with ExitStack() as es_d:
                pwp = es_d.enter_context(tc.tile_pool(name="pwp", bufs=3))
                ps_d = es_d.enter_context(
                    tc.tile_pool(name="ps_d", bufs=4, space="PSUM"))
                for opb in range(12):
                    wp_t = pwp.tile([128, 12, 128], bf16, tag="wp",
                                    name=f"wp{opb}")
                    wps_t = pwp.tile([4, 128], f32r, tag="wps",
                                     name=f"wps{opb}")
                    nc.sync.dma_start(wp_t[:], w_proj[opb])
                    nc.sync.dma_start(wps_t[:], w_proj_s[opb])
                    pr_ps = ps_d.tile([128, OWN], f32, tag="pr",
                                      name=f"prps{opb}")
                    for kb in range(12):
                        nc.tensor.matmul(pr_ps[:], wp_t[:, kb, :],
                                         attn[:, kb, :],
                                         start=(kb == 0), stop=False)
                    nc.tensor.matmul(pr_ps[:], wps_t[:], t_stat_one[:],
                                     start=False, stop=True)
                    nc.vector.tensor_tensor(xr1[:, opb, :], pr_ps[:],
                                            xr1[:, opb, :], op=ALU.add)
                    nc.vector.tensor_copy(r1r[:, opb, :], xr1[:, opb, :])
                if debug:
                    nc.sync.dma_start(dbg["d_r1"][:], xr1[:, 0, :])
            es_attn.close()

        # ---------------- phase E: LN2 stats over own tokens --------------
        stat2 = poolR1.tile([4, OWN], f32r, name="stat2")
        rstd2_bc = poolR1.tile([128, OWN], f32, name="rstd2_bc")
        with ExitStack() as es_e:
            pe = es_e.enter_context(tc.tile_pool(name="pe_sb", bufs=1))
            pdram2 = es_e.enter_context(
                tc.tile_pool(name="pdram2", bufs=1, space="DRAM"))
            pe_ps = es_e.enter_context(
                tc.tile_pool(name="pe_ps", bufs=2, space="PSUM"))
            sq2s = []
            for kb in range(12):
                sq2 = pe.tile([128, OWN], bf16, tag="sq2", name=f"sq2_{kb}", bufs=12)
                nc.scalar.activation(sq2[:], r1r[:, kb, :], AF.Square)
                sq2s.append(sq2)
            mu2_ps = pe_ps.tile([2, OWN], f32, tag="mu2", name="mu2")
            s2_ps = pe_ps.tile([1, OWN], f32, tag="s2", name="s2")
            for kb in range(12):
                lhs = t_ones_ab[:, 0:2] if kb < 6 else t_ones_ab[:, 2:4]
                nc.tensor.matmul(mu2_ps[:], lhs, r1r[:, kb, :],
                                 start=(kb == 0), stop=(kb == 11))
                nc.tensor.matmul(s2_ps[:], t_ones_s[:], sq2s[kb][:],
                                 start=(kb == 0), stop=(kb == 11))
            mu2_sb = pe.tile([2, OWN], f32, tag="emusb", name="emusb")
            mu2_fl = pe.tile([1, 2, OWN], f32, tag="emufl", name="emufl")
            var = pe.tile([1, OWN], f32, tag="evar", name="evar")
            lnv = pe.tile([1, OWN], f32, tag="elnv", name="elnv")
            s2_c = pe.tile([1, OWN], f32, tag="es2c", name="es2c")
            nc.vector.tensor_scalar(mu2_sb[:], mu2_ps[:], 1.0 / C, None,
                                    op0=ALU.mult)
            nc.vector.tensor_scalar(s2_c[:], s2_ps[:], 1.0 / C, None,
                                    op0=ALU.mult)
            nc.sync.dma_start(mu2_fl[:, 0, :], mu2_sb[0:1, :])
            nc.sync.dma_start(mu2_fl[:, 1, :], mu2_sb[1:2, :])
            sq_mu2 = pe.tile([1, 2, OWN], f32, tag="esqmu", name="esqmu")
            nc.vector.tensor_tensor(sq_mu2[:], mu2_fl[:], mu2_fl[:], op=ALU.mult)
            nc.vector.tensor_tensor(var[:], s2_c[:], sq_mu2[:, 0, :],
                                    op=ALU.subtract)
            nc.vector.tensor_tensor(var[:], var[:], sq_mu2[:, 1, :],
                                    op=ALU.subtract)
            nc.scalar.activation(lnv[:], var[:], AF.Ln, bias=t_eps[:])
            nc.vector.tensor_copy(stat2[0:2, :], mu2_sb[:])
            std2_row = pe.tile([1, OWN], f32r, tag="estd", name="estd")
            nc.scalar.activation(std2_row[:], lnv[:], AF.Exp, scale=0.5)
            nc.sync.dma_start(stat2[2:3, :], std2_row[:])
            rstd2_row = pe.tile([1, OWN], f32r, tag="ers", name="ers")
            nc.scalar.activation(rstd2_row[:], lnv[:], AF.Exp, scale=-0.5)
            rstd2_dram = pdram2.tile([1, OWN], f32, name="rstd2_dram")
            nc.sync.dma_start(rstd2_dram[:], rstd2_row[:].bitcast(f32))
            bcast2 = bass.AP(tensor=rstd2_dram.tensor, offset=rstd2_dram[:].offset,
                             ap=[[0, 128]] + rstd2_dram[:].ap[1:])
            nc.sync.dma_start(rstd2_bc[:], bcast2)
            if debug:
                nc.sync.dma_start(dbg["d_stat2"][:], stat2[:].bitcast(f32))

        # ---------------- phase F: MLP, single 512-token pass -------------
        with ExitStack() as es_f:
            pneg = es_f.enter_context(tc.tile_pool(name="pneg", bufs=1))
            r1neg = pneg.tile([128, 6, OWN], bf16, name="r1neg")
            for kb in range(6):
                nc.vector.tensor_scalar(r1neg[:, kb, :], r1r[:, 6 + kb, :],
                                        -1.0, None, op0=ALU.mult)
            ph = es_f.enter_context(tc.tile_pool(name="ph", bufs=1))
            phn = es_f.enter_context(tc.tile_pool(name="phn", bufs=1))
            pw1 = es_f.enter_context(tc.tile_pool(name="pw1", bufs=3))
            pw2 = es_f.enter_context(tc.tile_pool(name="pw2", bufs=3))
            pscf = es_f.enter_context(tc.tile_pool(name="pscf", bufs=4))
            pout = es_f.enter_context(tc.tile_pool(name="pout", bufs=2))
            ps_f = es_f.enter_context(
                tc.tile_pool(name="ps_f", bufs=4, space="PSUM"))
            h_t = ph.tile([128, 48, OWN], bf16, name="h_t")
            hn_t = phn.tile([128, 24, OWN], bf16, name="hn_t")
            for Cb in range(24):
                w1r_t = pw1.tile([128, 6, 128], bf16, tag="w1r",
                                 name=f"w1r{Cb}")
                w1i_t = pw1.tile([128, 6, 128], bf16, tag="w1i",
                                 name=f"w1i{Cb}")
                w1s_t = pw1.tile([4, 256], f32r, tag="w1s",
                                 name=f"w1s{Cb}")
                nc.sync.dma_start(w1r_t[:], w_fc1r[Cb])
                nc.sync.dma_start(w1i_t[:], w_fc1i[Cb])
                nc.sync.dma_start(w1s_t[:], w_fc1_s[Cb])
                hr_ps = ps_f.tile([128, OWN], f32, tag="fps",
                                  name=f"hrps{Cb}")
                hi_ps = ps_f.tile([128, OWN], f32, tag="fps",
                                  name=f"hips{Cb}")
                for kb in range(6):
                    st = (kb == 0)
                    nc.tensor.matmul(hr_ps[:], w1r_t[:, kb, :],
                                     r1r[:, kb, :], start=st, stop=False)
                    nc.tensor.matmul(hi_ps[:], w1i_t[:, kb, :],
                                     r1r[:, kb, :], start=st, stop=False)
                for kb in range(6):
                    nc.tensor.matmul(hr_ps[:], w1i_t[:, kb, :],
                                     r1neg[:, kb, :], start=False, stop=False)
                    nc.tensor.matmul(hi_ps[:], w1r_t[:, kb, :],
                                     r1r[:, 6 + kb, :], start=False,
                                     stop=False)
                nc.tensor.matmul(hr_ps[:], w1s_t[:, 0:128], stat2[:],
                                 start=False, stop=True)
                nc.tensor.matmul(hi_ps[:], w1s_t[:, 128:256], stat2[:],
                                 start=False, stop=True)
                gr = pscf.tile([128, OWN], f32, tag="g", name=f"gr{Cb}")
                gi = pscf.tile([128, OWN], f32, tag="g", name=f"gi{Cb}")
                nc.vector.tensor_tensor(gr[:], hr_ps[:], rstd2_bc[:],
                                        op=ALU.mult)
                nc.vector.tensor_tensor(gi[:], hi_ps[:], rstd2_bc[:],
                                        op=ALU.mult)
                nc.scalar.activation(h_t[:, Cb, :], gr[:], AF.Gelu)
                nc.scalar.activation(h_t[:, 24 + Cb, :], gi[:], AF.Gelu)
                nc.vector.tensor_scalar(hn_t[:, Cb, :], h_t[:, 24 + Cb, :],
                                        -1.0, None, op0=ALU.mult)
            if debug:
                h_dbg = pscf.tile([128, OWN], f32, tag="hdbg", name="hdbg")
                nc.vector.tensor_copy(h_dbg[:], h_t[:, 0, :])
                nc.sync.dma_start(dbg["d_h"][:], h_dbg[:])
            for j in range(6):
                w2r_t = pw2.tile([128, 24, 128], bf16, tag="w2",
                                 name=f"w2r{j}")
                w2i_t = pw2.tile([128, 24, 128], bf16, tag="w2",
                                 name=f"w2i{j}")
                w2s_t = pw2.tile([4, 256], f32r, tag="w2s",
                                 name=f"w2s{j}")
                nc.sync.dma_start(w2r_t[:], w_fc2r[j])
                nc.sync.dma_start(w2i_t[:], w_fc2i[j])
                nc.sync.dma_start(w2s_t[:], w_fc2_s[j])
                or_ps = ps_f.tile([128, OWN], f32, tag="fps",
                                  name=f"orps{j}")
                oi_ps = ps_f.tile([128, OWN], f32, tag="fps",
                                  name=f"oips{j}")
                for kb in range(24):
                    st = (kb == 0)
                    nc.tensor.matmul(or_ps[:], w2r_t[:, kb, :], h_t[:, kb, :],
                                     start=st, stop=False)
                    nc.tensor.matmul(oi_ps[:], w2i_t[:, kb, :], h_t[:, kb, :],
                                     start=st, stop=False)
                for kb in range(24):
                    nc.tensor.matmul(or_ps[:], w2i_t[:, kb, :], hn_t[:, kb, :],
                                     start=False, stop=False)
                    nc.tensor.matmul(oi_ps[:], w2r_t[:, kb, :],
                                     h_t[:, 24 + kb, :],
                                     start=False, stop=False)
                nc.tensor.matmul(or_ps[:], w2s_t[:, 0:128],
                                 t_stat_one[:], start=False, stop=True)
                nc.tensor.matmul(oi_ps[:], w2s_t[:, 128:256],
                                 t_stat_one[:], start=False, stop=True)
                o_r = pout.tile([128, OWN], f32, tag="o", name=f"or{j}")
                o_i = pout.tile([128, OWN], f32, tag="o", name=f"oi{j}")
                nc.vector.tensor_tensor(o_r[:], or_ps[:], xr1[:, j, :],
                                        op=ALU.add)
                nc.vector.tensor_tensor(o_i[:], oi_ps[:], xr1[:, 6 + j, :],
                                        op=ALU.add)
                nc.sync.dma_start(out_fm[j], o_r[:])
                nc.sync.dma_start(out_fm[6 + j], o_i[:])
    nc.compile()
    return nc


# --------------------------------------------------------------------------
# host side
# --------------------------------------------------------------------------

def _cx(a):
    return a[..., 0].astype(np.float64) + 1j * a[..., 1].astype(np.float64)


def _kcols(Wp, wsum, wb, plane, scale=1.0):
    """K-profile [1539, m] for output features with complex weight rows Wp
    [m, 768], LN fold sums wsum [m], bias-column wb [m]. K rows: xr(768),
    xi(768), mu_r, mu_i, std."""
    m = Wp.shape[0]
    out = np.zeros((1539, m), np.float64)
    if plane == "r":
        out[0:768] = Wp.real.T
        out[768:1536] = -Wp.imag.T
        out[1536] = -wsum.real
        out[1537] = wsum.imag
        out[1538] = wb.real
    else:
        out[0:768] = Wp.imag.T
        out[768:1536] = Wp.real.T
        out[1536] = -wsum.imag
        out[1537] = -wsum.real
        out[1538] = wb.imag
    return out * scale


def _bf(a):
    return np.ascontiguousarray(a).astype(BF16)


def _pmajor(a):
    """[12or6or24 kb, 128, n] -> [128, kb, n] partition-major contiguous."""
    return np.ascontiguousarray(np.transpose(a, (1, 0, 2)))


def _prep_weights(inputs):
    n1 = _cx(inputs["n1_w"]); b1 = _cx(inputs["n1_b"])
    n2 = _cx(inputs["n2_w"]); b2 = _cx(inputs["n2_b"])
    Wqkv = _cx(inputs["qkv_w"])          # [2304, 768]
    Wp = _cx(inputs["proj_w"])           # [768, 768]
    bp = _cx(inputs["proj_b"])           # [768]
    W1 = _cx(inputs["fc1_w"])            # [3072, 768]
    bf1 = _cx(inputs["fc1_b"])           # [3072]
    W2 = _cx(inputs["fc2_w"])            # [768, 3072]
    bf2 = _cx(inputs["fc2_b"])           # [768]

    d = {}
    # ---- qkv (LN1-folded) ----
    Wq, Wk, Wv = Wqkv[0:768], Wqkv[768:1536], Wqkv[1536:2304]

    def fold1(W):
        Wf = W * n1[None, :]
        return Wf, Wf.sum(1), W @ b1

    w_qkv = np.zeros((H, 128, 12, 384), BF16)
    w_qkv_s = np.zeros((H, 4, 384), np.float32)
    for h in range(H):
        rows = slice(h * DH, (h + 1) * DH)
        Qf, Qs, Qb = fold1(Wq[rows])
        Kf, Ks, Kb_ = fold1(Wk[rows])
        q1 = np.hstack([_kcols(Qf, Qs, Qb, "r", SCALE),
                        _kcols(Qf, Qs, Qb, "i", -SCALE)])
        q3 = np.hstack([_kcols(Qf, Qs, Qb, "i", SCALE),
                        _kcols(Qf, Qs, Qb, "r", SCALE)])
        kk = np.hstack([_kcols(Kf, Ks, Kb_, "r"), _kcols(Kf, Ks, Kb_, "i")])
        blk = np.hstack([q1, q3, kk]).astype(np.float32)       # [1539, 384]
        w_qkv[h] = _bf(_pmajor(blk[0:1536].reshape(12, 128, 384)))
        w_qkv_s[h, 0:3] = blk[1536:1539]
    d["w_qkv"] = w_qkv
    d["w_qkv_s"] = round_fp32r(w_qkv_s)

    # ---- v (LN1-folded), rhs layout; cols: pair*256+slot*128+plane*64+dh
    wv_full = np.zeros((1539, 1536), np.float64)
    for h in range(H):
        rows = slice(h * DH, (h + 1) * DH)
        Vf, Vs, Vb = fold1(Wv[rows])
        base = h * 128
        wv_full[:, base:base + 64] = _kcols(Vf, Vs, Vb, "r")
        wv_full[:, base + 64:base + 128] = _kcols(Vf, Vs, Vb, "i")
    w_v = np.zeros((6, 128, 12, 256), BF16)
    for pair in range(6):
        csl = slice(pair * 256, pair * 256 + 256)
        w_v[pair] = _bf(_pmajor(wv_full[0:1536, csl].reshape(12, 128, 256)))
    d["w_v"] = w_v
    wvs = np.zeros((4, 1536), np.float32)
    wvs[0:3] = wv_full[1536:1539]
    d["w_v_s"] = round_fp32r(wvs)

    # ---- proj (plain + bias); K rows = attn features: per head [a_r(64); a_i(64)]
    w_proj = np.zeros((12, 128, 12, 128), BF16)
    w_proj_s = np.zeros((12, 4, 128), np.float32)
    for opb in range(12):
        plane = "r" if opb < 6 else "i"
        orow = slice((opb % 6) * 128, (opb % 6) * 128 + 128)
        Wpo = Wp[orow]                               # [128, 768] complex
        prof = np.zeros((1536, 128), np.float64)
        for hh in range(H):
            cols = slice(hh * DH, (hh + 1) * DH)
            if plane == "r":
                prof[hh * 128:hh * 128 + 64] = Wpo.real[:, cols].T
                prof[hh * 128 + 64:hh * 128 + 128] = -Wpo.imag[:, cols].T
            else:
                prof[hh * 128:hh * 128 + 64] = Wpo.imag[:, cols].T
                prof[hh * 128 + 64:hh * 128 + 128] = Wpo.real[:, cols].T
        w_proj[opb] = _bf(_pmajor(prof.reshape(12, 128, 128)))
        w_proj_s[opb, 0] = (bp.real if plane == "r" else bp.imag)[orow]
    d["w_proj"] = w_proj
    d["w_proj_s"] = round_fp32r(w_proj_s)

    # ---- fc1 (LN2-folded, shared-tile form) ----
    W1f = W1 * n2[None, :]
    W1s = W1f.sum(1)
    W1b = W1 @ b2 + bf1
    w_fc1r = np.zeros((24, 128, 6, 128), BF16)
    w_fc1i = np.zeros((24, 128, 6, 128), BF16)
    w_fc1_s = np.zeros((24, 4, 256), np.float32)
    for Cb in range(24):
        orow = slice(Cb * 128, (Cb + 1) * 128)
        tr = np.zeros((6, 128, 128), np.float64)
        ti = np.zeros((6, 128, 128), np.float64)
        for kb in range(6):
            icol = slice(kb * 128, (kb + 1) * 128)
            tr[kb] = W1f.real[orow, icol].T
            ti[kb] = W1f.imag[orow, icol].T
        w_fc1r[Cb] = _bf(_pmajor(tr))
        w_fc1i[Cb] = _bf(_pmajor(ti))
        w_fc1_s[Cb, 0, 0:128] = -W1s.real[orow]
        w_fc1_s[Cb, 1, 0:128] = W1s.imag[orow]
        w_fc1_s[Cb, 2, 0:128] = W1b.real[orow]
        w_fc1_s[Cb, 0, 128:256] = -W1s.imag[orow]
        w_fc1_s[Cb, 1, 128:256] = -W1s.real[orow]
        w_fc1_s[Cb, 2, 128:256] = W1b.imag[orow]
    d["w_fc1r"] = w_fc1r
    d["w_fc1i"] = w_fc1i
    d["w_fc1_s"] = round_fp32r(w_fc1_s)

    # ---- fc2 (plain + bias) ----
    w_fc2r = np.zeros((6, 128, 24, 128), BF16)
    w_fc2i = np.zeros((6, 128, 24, 128), BF16)
    w_fc2_s = np.zeros((6, 4, 256), np.float32)
    for j in range(6):
        orow = slice(j * 128, (j + 1) * 128)
        tr = np.zeros((24, 128, 128), np.float64)
        ti = np.zeros((24, 128, 128), np.float64)
        for kb in range(24):
            icol = slice(kb * 128, (kb + 1) * 128)
            tr[kb] = W2.real[orow, icol].T
            ti[kb] = W2.imag[orow, icol].T
        w_fc2r[j] = _bf(_pmajor(tr))
        w_fc2i[j] = _bf(_pmajor(ti))
        w_fc2_s[j, 0, 0:128] = bf2.real[orow]
        w_fc2_s[j, 0, 128:256] = bf2.imag[orow]
    d["w_fc2r"] = w_fc2r
    d["w_fc2i"] = w_fc2i
    d["w_fc2_s"] = round_fp32r(w_fc2_s)

    # ---- consts ----
    d["ones_col"] = np.ones((128, 1), BF16)
    oab = np.zeros((128, 4), np.float32)
    oab[:, 0] = 1.0
    oab[:, 3] = 1.0
    d["ones_ab"] = oab.astype(BF16)
    d["ones_s"] = np.ones((128, 1), BF16)
    so = np.zeros((4, OWN), np.float32)
    so[0] = 1.0
    d["stat_one"] = so
    d["ident8"] = np.eye(8, dtype=np.float32)
    return d


_NC_CACHE = {}


def kernel(**inputs):
    debug = bool(inputs.pop("_debug", False))
    if debug not in _NC_CACHE:
        _NC_CACHE[debug] = build_nc(debug=debug)
    nc = _NC_CACHE[debug]

    shared = _prep_weights(inputs)
    x = np.asarray(inputs["x"], np.float32)          # [B, N, C, 2]

    in_maps = []
    for c in range(NCORES):
        b, half = divmod(c, 2)
        xr_ = x[b, :, :, 0].T                        # [768, 1024]
        xi_ = x[b, :, :, 1].T
        stack = np.concatenate([xr_, xi_], 0)        # [1536, 1024]
        if half == 1:
            stack = np.concatenate([stack[:, OWN:], stack[:, :OWN]], 1)
        m = dict(shared)
        m["x_r"] = np.ascontiguousarray(
            stack.reshape(12, 128, N).transpose(1, 0, 2)).astype(BF16)
        m["x_own"] = np.ascontiguousarray(
            stack[:, 0:OWN].reshape(12, 128, OWN).transpose(1, 0, 2))
        in_maps.append(m)

    res = run_bass_kernel_spmd(nc, in_maps, list(range(NCORES)))
    out = np.empty((B, N, C, 2), np.float32)
    for c in range(NCORES):
        b, half = divmod(c, 2)
        o = res.results[c]["out_fm"]                 # [12, 128, OWN]
        sl = slice(half * OWN, half * OWN + OWN)
        out[b, sl, :, 0] = o[0:6].reshape(768, OWN).T
        out[b, sl, :, 1] = o[6:12].reshape(768, OWN).T
    if debug:
        return out, res
    return out


# revision 8
# speedup vs baseline: 1.4672x; 1.0825x over previous
"""Complex transformer block (LN->attn->LN->MLP, complex arithmetic) on 8 TRN2 cores.

Sharding: core c handles (batch b = c//2, sequence half = c%2). No collectives:
each core computes K/V over the full 1024-token sequence of its batch (the only
duplicated work) and queries/MLP over its own 512 tokens.

Layout: activations are feature-major [feature partition-blocks, tokens].
Complex tensors are realified as separate real/imag feature planes. LayerNorm
is fused into the following matmul: per-token stats (mu_r, mu_i, std) are
appended as 3 extra contraction rows with matching weight columns, and the
per-token rstd is applied by the PSUM-eviction multiply. Attention scores are
computed transposed ([t2, t1]) so softmax sums reduce via ones-matmuls, and V
is produced pre-transposed by swapping matmul operands.

All weights are stored bf16 host-side in the exact SBUF layout (partition-major)
so every weight DMA is a single fully-contiguous transfer, and bf16 stationaries
get the fast-weight-load (FWL) path on the PE. Activations stay float32r except
k/vt/et (matmul stationaries or moving operands where bf16 keeps full PE rate).
"""
import sys
sys.path.insert(0, "/opt/trn_rl_repo")

from contextlib import ExitStack

import ml_dtypes
import numpy as np

import concourse.bacc as bacc
import concourse.bass as bass
import concourse.mybir as mybir
import concourse.tile as tile
from concourse.bass_utils import run_bass_kernel_spmd

# Prefer the table set that covers the whole softmax chain (square+ln+exp)
# so the greedy act-table-load pass doesn't thrash sets on every block.
_orig_get_tables = bacc.get_activation_tables


def _reordered_tables(arch):
    # Keep canonical order/indices (walrus resolves act_func_set_id by
    # position) but blank every set except the two we want, so the greedy
    # table-load pass can't thrash between sets per softmax block.
    t = _orig_get_tables(arch)
    keep = {"natural_log_exp_and_others", "gelu_and_others"}
    return {k: (v if k in keep else set()) for k, v in t.items()}


bacc.get_activation_tables = _reordered_tables

dt = mybir.dt
AF = mybir.ActivationFunctionType
ALU = mybir.AluOpType
BF16 = ml_dtypes.bfloat16

B, N, C, H, DH, HID = 4, 1024, 768, 12, 64, 3072
NCORES = 8
OWN = 512          # tokens per core
KB = C // 128      # 6 feature pblocks per plane
SCALE = DH ** -0.5
EPS = 1e-5


def round_fp32r(x):
    b = np.ascontiguousarray(x, dtype=np.float32).view(np.uint32)
    lsb = (b >> np.uint32(12)) & np.uint32(1)
    return ((b + np.uint32(0x7FF) + lsb) & np.uint32(0xFFFFF000)).view(np.float32)


# --------------------------------------------------------------------------
# device program
# --------------------------------------------------------------------------

def build_nc(debug=False):
    nc = bacc.Bacc(trn_type="TRN2", target_bir_lowering=False)
    f32 = dt.float32
    f32r = dt.float32r
    bf16 = dt.bfloat16

    # ---- DRAM I/O ----
    x_r = nc.dram_tensor("x_r", [128, 12, N], bf16, kind="ExternalInput")
    x_own = nc.dram_tensor("x_own", [128, 12, OWN], f32, kind="ExternalInput")
    w_qkv = nc.dram_tensor("w_qkv", [H, 128, 12, 384], bf16, kind="ExternalInput")
    w_qkv_s = nc.dram_tensor("w_qkv_s", [H, 4, 384], f32r, kind="ExternalInput")
    w_v = nc.dram_tensor("w_v", [6, 128, 12, 256], bf16, kind="ExternalInput")
    w_v_s = nc.dram_tensor("w_v_s", [4, 1536], f32r, kind="ExternalInput")
    w_proj = nc.dram_tensor("w_proj", [12, 128, 12, 128], bf16, kind="ExternalInput")
    w_proj_s = nc.dram_tensor("w_proj_s", [12, 4, 128], f32r, kind="ExternalInput")
    w_fc1r = nc.dram_tensor("w_fc1r", [24, 128, 6, 128], bf16, kind="ExternalInput")
    w_fc1i = nc.dram_tensor("w_fc1i", [24, 128, 6, 128], bf16, kind="ExternalInput")
    w_fc1in = nc.dram_tensor("w_fc1in", [24, 128, 6, 128], bf16, kind="ExternalInput")
    w_fc1_s = nc.dram_tensor("w_fc1_s", [24, 4, 256], f32r, kind="ExternalInput")
    w_fc2r = nc.dram_tensor("w_fc2r", [6, 128, 24, 128], bf16, kind="ExternalInput")
    w_fc2i = nc.dram_tensor("w_fc2i", [6, 128, 24, 128], bf16, kind="ExternalInput")
    w_fc2in = nc.dram_tensor("w_fc2in", [6, 128, 24, 128], bf16, kind="ExternalInput")
    w_fc2_s = nc.dram_tensor("w_fc2_s", [6, 4, 256], f32r, kind="ExternalInput")
    ones_col = nc.dram_tensor("ones_col", [128, 1], bf16, kind="ExternalInput")
    ones_ab = nc.dram_tensor("ones_ab", [128, 4], bf16, kind="ExternalInput")
    # ones_ab cols: [1/C, 0] (A: xr-plane mu), [0, 1/C] (B: xi-plane mu)
    ones_s = nc.dram_tensor("ones_s", [128, 1], bf16, kind="ExternalInput")
    stat_one = nc.dram_tensor("stat_one", [4, OWN], f32r, kind="ExternalInput")
    ident8 = nc.dram_tensor("ident8", [8, 8], f32r, kind="ExternalInput")

    out_fm = nc.dram_tensor("out_fm", [12, 128, OWN], f32, kind="ExternalOutput")
    dbg = {}
    if debug:
        for nm, shp in [
            ("d_stat1", [4, N]), ("d_q1", [128, OWN]), ("d_k", [128, N]),
            ("d_vt", [128, 8, 256]), ("d_exp", [128, OWN]), ("d_den", [1, OWN]),
            ("d_attn", [128, OWN]), ("d_r1", [128, OWN]), ("d_stat2", [4, OWN]),
            ("d_h", [128, OWN]), ("d_S", [1, 512]), ("d_var", [1, 512]),
            ("d_mufl", [1, 2, 512]), ("d_sq0", [128, 512]),
            ("d_rstdT", [128, 8]),
        ]:
            dbg[nm] = nc.dram_tensor(nm, shp, f32, kind="ExternalOutput")

    with tile.TileContext(nc) as tc, ExitStack() as top:
        consts = top.enter_context(tc.tile_pool(name="consts", bufs=1))
        t_ones_col = consts.tile([128, 1], bf16)
        t_ones_ab = consts.tile([128, 4], bf16)
        t_ones_s = consts.tile([128, 1], bf16)
        t_stat_one = consts.tile([4, OWN], f32r)
        t_id8 = consts.tile([8, 8], f32r)
        t_eps = consts.tile([1, 1], f32)
        nc.sync.dma_start(t_ones_col[:], ones_col[:])
        nc.sync.dma_start(t_ones_ab[:], ones_ab[:])
        nc.sync.dma_start(t_ones_s[:], ones_s[:])
        nc.sync.dma_start(t_stat_one[:], stat_one[:])
        nc.sync.dma_start(t_id8[:], ident8[:])
        nc.vector.memset(t_eps[:], EPS)

        poolR1 = top.enter_context(tc.tile_pool(name="poolR1", bufs=1))
        xr1 = poolR1.tile([128, 12, OWN], f32, name="xr1")

        with ExitStack() as es_x:
            poolX = es_x.enter_context(tc.tile_pool(name="poolX", bufs=1))
            xrb = poolX.tile([128, 12, N], bf16, name="xrb")
            pdram = es_x.enter_context(
                tc.tile_pool(name="pdram", bufs=1, space="DRAM"))
            rstd_dram = pdram.tile([1, N], f32, name="rstd_dram")
            stat1s = [poolX.tile([4, 512], f32r, name=f"stat1_{ch}")
                      for ch in range(2)]
            rstd_bc1s = [poolX.tile([128, 512], f32, name=f"rstd_bc1_{ch}")
                         for ch in range(2)]
            rstdT = poolX.tile([128, 8], f32, name="rstdT")
            for kb in range(12):
                nc.sync.dma_start(xrb[:, kb, :], x_r[:, kb, :])

            # ---------------- phase A: LN1 stats over full sequence --------
            with ExitStack() as es_a:
                pa = es_a.enter_context(tc.tile_pool(name="pa_sb", bufs=3))
                pa_ps = es_a.enter_context(
                    tc.tile_pool(name="pa_ps", bufs=2, space="PSUM"))
                pa_sc = es_a.enter_context(tc.tile_pool(name="pa_sc", bufs=2))
                mu_pss = [pa_ps.tile([2, 512], f32, tag=f"mu{ch}",
                                     name=f"mu{ch}", bufs=1) for ch in range(2)]
                s_pss = [pa_ps.tile([1, 512], f32, tag=f"s{ch}",
                                    name=f"s{ch}", bufs=1) for ch in range(2)]
                for kb in range(12):
                    sq = pa.tile([128, N], bf16, tag="sq", name=f"sq{kb}")
                    nc.scalar.activation(sq[:], xrb[:, kb, :], AF.Square)
                    lhs = t_ones_ab[:, 0:2] if kb < 6 else t_ones_ab[:, 2:4]
                    for ch in range(2):
                        sl = slice(ch * 512, ch * 512 + 512)
                        nc.tensor.matmul(mu_pss[ch][:], lhs, xrb[:, kb, sl],
                                         start=(kb == 0), stop=(kb == 11))
                        nc.tensor.matmul(s_pss[ch][:], t_ones_s[:], sq[:, sl],
                                         start=(kb == 0), stop=(kb == 11))
                for ch in range(2):
                    sl = slice(ch * 512, ch * 512 + 512)
                    mu_ps = mu_pss[ch]
                    s_ps = s_pss[ch]
                    # var = S - mu_r^2 - mu_i^2 ; std = exp(.5 ln(var+eps))
                    mu_sb = pa_sc.tile([2, 512], f32, tag="musb", name=f"musb{ch}")
                    mu_fl = pa_sc.tile([1, 2, 512], f32, tag="mufl", name=f"mufl{ch}")
                    var = pa_sc.tile([1, 512], f32, tag="var", name=f"var{ch}")
                    lnv = pa_sc.tile([1, 512], f32, tag="lnv", name=f"lnv{ch}")
                    s_c = pa_sc.tile([1, 512], f32, tag="sc_", name=f"sc_{ch}")
                    nc.vector.tensor_scalar(mu_sb[:], mu_ps[:], 1.0 / C, None,
                                            op0=ALU.mult)
                    nc.vector.tensor_scalar(s_c[:], s_ps[:], 1.0 / C, None,
                                            op0=ALU.mult)
                    nc.sync.dma_start(mu_fl[:, 0, :], mu_sb[0:1, :])
                    nc.sync.dma_start(mu_fl[:, 1, :], mu_sb[1:2, :])
                    sq_mu = pa_sc.tile([1, 2, 512], f32, tag="sqmu", name=f"sqmu{ch}")
                    nc.vector.tensor_tensor(sq_mu[:], mu_fl[:], mu_fl[:],
                                            op=ALU.mult)
                    nc.vector.tensor_tensor(var[:], s_c[:], sq_mu[:, 0, :],
                                            op=ALU.subtract)
                    nc.vector.tensor_tensor(var[:], var[:], sq_mu[:, 1, :],
                                            op=ALU.subtract)
                    nc.scalar.activation(lnv[:], var[:], AF.Ln, bias=t_eps[:])
                    if debug and ch == 0:
                        nc.sync.dma_start(dbg["d_var"][:], var[:])
                        nc.sync.dma_start(dbg["d_mufl"][:], mu_fl[:])
                        s_sb_dbg = pa_sc.tile([1, 512], f32, tag="sdbg",
                                              name="sdbg")
                        nc.vector.tensor_copy(s_sb_dbg[:], s_ps[:])
                        nc.sync.dma_start(dbg["d_S"][:], s_sb_dbg[:])
                    # stats rows: 0=mu_r 1=mu_i 2=std
                    nc.vector.tensor_copy(stat1s[ch][0:2, :], mu_sb[:])
                    std_row = pa_sc.tile([1, 512], f32r, tag="stdr", name=f"stdr{ch}")
                    nc.scalar.activation(std_row[:], lnv[:], AF.Exp, scale=0.5)
                    nc.sync.dma_start(stat1s[ch][2:3, :], std_row[:])
                    rstd_row = pa_sc.tile([1, 512], f32r, tag="rst", name=f"rst{ch}")
                    nc.scalar.activation(rstd_row[:], lnv[:], AF.Exp, scale=-0.5)
                    nc.sync.dma_start(rstd_dram[:, sl], rstd_row[:].bitcast(f32))
                    bcast = bass.AP(tensor=rstd_dram.tensor,
                                    offset=rstd_dram[:, sl].offset,
                                    ap=[[0, 128]] + rstd_dram[:, sl].ap[1:])
                    nc.sync.dma_start(rstd_bc1s[ch][:], bcast)
                # rstd transposed: rstdT[p, t2b] = rstd[t2b*128 + p]
                rstd8 = pa_sc.tile([8, 128], f32, tag="r8", name="rstd8")
                nc.sync.dma_start(
                    rstd8[:], rstd_dram[:].rearrange("o (a b) -> (o a) b", a=8))
                rstdT_ps = pa_ps.tile([128, 8], f32, tag="rtps", name="rtps")
                nc.tensor.transpose(rstdT_ps[:], rstd8[:], t_id8[:].bitcast(f32))
                nc.vector.tensor_copy(rstdT[:], rstdT_ps[:])
                if debug:
                    nc.sync.dma_start(dbg["d_stat1"][:, 0:512],
                                      stat1s[0][:].bitcast(f32))
                    nc.sync.dma_start(dbg["d_stat1"][:, 512:N],
                                      stat1s[1][:].bitcast(f32))
                    nc.sync.dma_start(dbg["d_rstdT"][:], rstdT[:])

            # ---------------- phase BC: qkv + attention per head ----------
            es_attn = ExitStack()
            attnp = es_attn.enter_context(tc.tile_pool(name="attnp", bufs=1))
            attn = attnp.tile([128, 12, OWN], bf16, name="attn")
            es_b = ExitStack()
            pq = es_b.enter_context(tc.tile_pool(name="pq", bufs=1))
            pk = es_b.enter_context(tc.tile_pool(name="pk", bufs=1))
            pvt = es_b.enter_context(tc.tile_pool(name="pvt", bufs=2))
            pwv = es_b.enter_context(tc.tile_pool(name="pwv", bufs=1))
            pwq = es_b.enter_context(tc.tile_pool(name="pwq", bufs=2))
            pet = es_b.enter_context(tc.tile_pool(name="pet", bufs=6))
            psc = es_b.enter_context(tc.tile_pool(name="psc", bufs=6))
            prd = es_b.enter_context(tc.tile_pool(name="prd", bufs=2))
            ps_rot = es_b.enter_context(
                tc.tile_pool(name="ps_rot", bufs=2, space="PSUM"))
            ps_sc = es_b.enter_context(
                tc.tile_pool(name="ps_sc", bufs=2, space="PSUM"))
            ps_acc = es_b.enter_context(
                tc.tile_pool(name="ps_acc", bufs=2, space="PSUM"))
            pdram_rd = es_b.enter_context(
                tc.tile_pool(name="pdram_rd", bufs=2, space="DRAM"))
            vt_pair = None
            et_fifo = []
            acc_ps = {}
            LAG = 4

            def emit_avden(ent):
                h2, t2b2, et2, vt2 = ent
                slot2 = h2 % 2
                if t2b2 == 0:
                    acc_ps[h2] = (
                        ps_acc.tile([128, OWN], f32, tag="av", name=f"av{h2}",
                                    bufs=1),
                        ps_acc.tile([1, OWN], f32, tag="den", name=f"den{h2}",
                                    bufs=1),
                    )
                av2, den2 = acc_ps[h2]
                nc.tensor.matmul(den2[:], t_ones_col[:], et2,
                                 start=(t2b2 == 0), stop=(t2b2 == 7))
                dsl2 = slice(slot2 * 128, slot2 * 128 + 128)
                nc.tensor.matmul(av2[:], vt2[:, t2b2, dsl2], et2,
                                 start=(t2b2 == 0), stop=(t2b2 == 7))
                if t2b2 == 7:
                    den_sb = prd.tile([1, OWN], f32, tag="den_sb",
                                      name=f"dsb{h2}", bufs=1)
                    nc.vector.tensor_copy(den_sb[:], den2[:])
                    den_dram = pdram_rd.tile([1, OWN], f32, tag="dend",
                                             name=f"dend{h2}")
                    nc.sync.dma_start(den_dram[:], den_sb[:])
                    den_sp = prd.tile([128, 4], f32, tag="den_sp",
                                      name=f"dsp{h2}", bufs=1)
                    nc.sync.dma_start(
                        den_sp[:],
                        den_dram[:].rearrange("o (a b) -> (o a) b", a=128))
                    rd_sp = prd.tile([128, 4], f32, tag="rd_sp",
                                     name=f"rsp{h2}", bufs=1)
                    nc.vector.reciprocal(rd_sp[:], den_sp[:])
                    rd_dram = pdram_rd.tile([1, OWN], f32, tag="rdd",
                                            name=f"rdd{h2}")
                    nc.sync.dma_start(
                        rd_dram[:].rearrange("o (a b) -> (o a) b", a=128),
                        rd_sp[:])
                    rd_bc = prd.tile([128, OWN], f32, tag="rd_bc",
                                     name=f"rdbc{h2}", bufs=1)
                    rd_bcast_ap = bass.AP(tensor=rd_dram.tensor,
                                          offset=rd_dram[:].offset,
                                          ap=[[0, 128]] + rd_dram[:].ap[1:])
                    nc.sync.dma_start(rd_bc[:], rd_bcast_ap)
                    nc.vector.tensor_tensor(attn[:, h2, :], av2[:], rd_bc[:],
                                            op=ALU.mult)
                    del acc_ps[h2]
                    if debug and h2 == 0:
                        nc.sync.dma_start(dbg["d_den"][:], den_sb[:])
                        nc.sync.dma_start(dbg["d_attn"][:],
                                          attn[:, 0, :].bitcast(f32))

            for h in range(H):
                pair, slot = divmod(h, 2)
                # qkv for head h: q1=[q_r;-q_i], q3=[q_i;q_r], k=[k_r;k_i]
                q_t = pq.tile([128, 2, OWN], bf16, tag="q", name=f"q{h}")
                k_t = pk.tile([128, N], bf16, tag="k", name=f"k{h}")
                wqkv_t = pwq.tile([128, 12, 384], bf16, tag="wqkv",
                                  name=f"wqkv{h}")
                wqs_t = pwq.tile([4, 384], f32r, tag="wqs", name=f"wqs{h}")
                nc.sync.dma_start(wqkv_t[:], w_qkv[h])
                nc.sync.dma_start(wqs_t[:], w_qkv_s[h])
                q1_ps = ps_rot.tile([128, OWN], f32, tag="rot", name=f"q1ps{h}")
                q3_ps = ps_rot.tile([128, OWN], f32, tag="rot", name=f"q3ps{h}")
                for kb in range(12):
                    st = (kb == 0)
                    nc.tensor.matmul(q1_ps[:], wqkv_t[:, kb, 0:128],
                                     xrb[:, kb, 0:OWN], start=st, stop=False)
                    nc.tensor.matmul(q3_ps[:], wqkv_t[:, kb, 128:256],
                                     xrb[:, kb, 0:OWN], start=st, stop=False)
                nc.tensor.matmul(q1_ps[:], wqs_t[:, 0:128], stat1s[0][:],
                                 start=False, stop=True)
                nc.tensor.matmul(q3_ps[:], wqs_t[:, 128:256], stat1s[0][:],
                                 start=False, stop=True)
                nc.vector.tensor_tensor(q_t[:, 0, :], q1_ps[:],
                                        rstd_bc1s[0][:], op=ALU.mult)
                nc.vector.tensor_tensor(q_t[:, 1, :], q3_ps[:],
                                        rstd_bc1s[0][:], op=ALU.mult)
                k0_ps = ps_rot.tile([128, 512], f32, tag="rot", name=f"k0ps{h}")
                k1_ps = ps_rot.tile([128, 512], f32, tag="rot", name=f"k1ps{h}")
                for kb in range(12):
                    st = (kb == 0)
                    nc.tensor.matmul(k0_ps[:], wqkv_t[:, kb, 256:384],
                                     xrb[:, kb, 0:512], start=st, stop=False)
                    nc.tensor.matmul(k1_ps[:], wqkv_t[:, kb, 256:384],
                                     xrb[:, kb, 512:N], start=st, stop=False)
                nc.tensor.matmul(k0_ps[:], wqs_t[:, 256:384], stat1s[0][:],
                                 start=False, stop=True)
                nc.tensor.matmul(k1_ps[:], wqs_t[:, 256:384], stat1s[1][:],
                                 start=False, stop=True)
                nc.vector.tensor_tensor(k_t[:, 0:512], k0_ps[:],
                                        rstd_bc1s[0][:], op=ALU.mult)
                nc.vector.tensor_tensor(k_t[:, 512:N], k1_ps[:],
                                        rstd_bc1s[1][:], op=ALU.mult)
                if debug and h == 0:
                    nc.sync.dma_start(dbg["d_q1"][:],
                                      q_t[:, 0, :].bitcast(f32))
                    k_dbg = prd.tile([128, N], f32, tag="kdbg", name="kdbg")
                    nc.vector.tensor_copy(k_dbg[:], k_t[:])
                    nc.sync.dma_start(dbg["d_k"][:], k_dbg[:])
                if slot == 0:
                    # V^T for this head pair: [t2, d] via swapped operands
                    wv_t = pwv.tile([128, 12, 256], bf16, tag="wv",
                                    name=f"wv{pair}")
                    wv_s = pwv.tile([4, 256], f32r, tag="wvs",
                                    name=f"wvs{pair}")
                    csl = slice(pair * 256, pair * 256 + 256)
                    nc.sync.dma_start(wv_t[:], w_v[pair])
                    nc.sync.dma_start(wv_s[:], w_v_s[:, csl])
                    vt_pair = pvt.tile([128, 8, 256], bf16, tag="vt",
                                       name=f"vt{pair}")
                    for t2b in range(8):
                        t2s = slice(t2b * 128, t2b * 128 + 128)
                        vt_ps = ps_rot.tile([128, 256], f32, tag="rot",
                                            name=f"vtps{pair}_{t2b}")
                        for kb in range(12):
                            nc.tensor.matmul(vt_ps[:], xrb[:, kb, t2s],
                                             wv_t[:, kb, :],
                                             start=(kb == 0), stop=False)
                        st1 = stat1s[t2b // 4]
                        t2l = slice((t2b % 4) * 128, (t2b % 4) * 128 + 128)
                        nc.tensor.matmul(vt_ps[:], st1[:, t2l], wv_s[:],
                                         start=False, stop=True)
                        nc.vector.tensor_scalar(
                            vt_pair[:, t2b, :], vt_ps[:],
                            rstdT[:, t2b:t2b + 1], None, op0=ALU.mult)
                    if debug and pair == 0:
                        vt_dbg = prd.tile([128, 8, 256], f32, tag="vtdbg",
                                          name="vtdbg")
                        nc.vector.tensor_copy(vt_dbg[:], vt_pair[:])
                        nc.sync.dma_start(dbg["d_vt"][:], vt_dbg[:])
                # scores + exp chain, batched over block pairs;
                # den/av matmuls lag by LAG sub-blocks
                for t2p in range(4):
                    t2s0 = slice(t2p * 256, t2p * 256 + 128)
                    t2s1 = slice(t2p * 256 + 128, t2p * 256 + 256)
                    sr_pair = ps_sc.tile([128, 2, OWN], f32, tag="scp",
                                         name=f"srp{h}_{t2p}")
                    si_pair = ps_sc.tile([128, 2, OWN], f32, tag="scp",
                                         name=f"sip{h}_{t2p}")
                    nc.tensor.matmul(sr_pair[:, 0, :], k_t[:, t2s0],
                                     q_t[:, 0, :], start=True, stop=True)
                    nc.tensor.matmul(si_pair[:, 0, :], k_t[:, t2s0],
                                     q_t[:, 1, :], start=True, stop=True)
                    nc.tensor.matmul(sr_pair[:, 1, :], k_t[:, t2s1],
                                     q_t[:, 0, :], start=True, stop=True)
                    nc.tensor.matmul(si_pair[:, 1, :], k_t[:, t2s1],
                                     q_t[:, 1, :], start=True, stop=True)
                    sqr = psc.tile([128, 2, OWN], f32, tag="sqr",
                                   name=f"sqr{h}_{t2p}")
                    sqi = psc.tile([128, 2, OWN], f32, tag="sqi",
                                   name=f"sqi{h}_{t2p}")
                    nc.scalar.activation(sqr[:], sr_pair[:], AF.Square)
                    nc.scalar.activation(sqi[:], si_pair[:], AF.Square)
                    # in-place chain on sqr: m2 -> ln -> 0.5ln -> mag -> exp
                    nc.vector.tensor_tensor(sqr[:], sqr[:], sqi[:], op=ALU.add)
                    nc.scalar.activation(sqr[:], sqr[:], AF.Ln)
                    nc.scalar.activation(sqr[:], sqr[:], AF.Exp, scale=0.5)
                    et = pet.tile([128, 2, OWN], bf16, tag="et",
                                  name=f"et{h}_{t2p}")
                    nc.scalar.activation(et[:], sqr[:], AF.Exp)
                    if debug and h == 0 and t2p == 0:
                        et_dbg = prd.tile([128, OWN], f32, tag="etdbg",
                                          name="etdbg")
                        nc.vector.tensor_copy(et_dbg[:], et[:, 0, :])
                        nc.sync.dma_start(dbg["d_exp"][:], et_dbg[:])
                    for sub in range(2):
                        et_fifo.append((h, t2p * 2 + sub, et[:, sub, :],
                                        vt_pair))
                        while len(et_fifo) > LAG:
                            emit_avden(et_fifo.pop(0))
            for ent in et_fifo:
                emit_avden(ent)
            et_fifo.clear()
            es_b.close()

            # ------------- phase D: proj + residual --------------------
            nc.sync.dma_start(xr1[:], x_own[:])
            r1r = poolR1.tile([128, 12, OWN], bf16, name="r1r")
            with ExitStack() as es_d:
                pwp = es_d.enter_context(tc.tile_pool(name="pwp", bufs=3))
                ps_d = es_d.enter_context(
                    tc.tile_pool(name="ps_d", bufs=4, space="PSUM"))
                for opb in range(12):
                    wp_t = pwp.tile([128, 12, 128], bf16, tag="wp",
                                    name=f"wp{opb}")
                    wps_t = pwp.tile([4, 128], f32r, tag="wps",
                                     name=f"wps{opb}")
                    nc.sync.dma_start(wp_t[:], w_proj[opb])
                    nc.sync.dma_start(wps_t[:], w_proj_s[opb])
                    pr_ps = ps_d.tile([128, OWN], f32, tag="pr",
                                      name=f"prps{opb}")
                    for kb in range(12):
                        nc.tensor.matmul(pr_ps[:], wp_t[:, kb, :],
                                         attn[:, kb, :],
                                         start=(kb == 0), stop=False)
                    nc.tensor.matmul(pr_ps[:], wps_t[:], t_stat_one[:],
                                     start=False, stop=True)
                    nc.vector.tensor_tensor(xr1[:, opb, :], pr_ps[:],
                                            xr1[:, opb, :], op=ALU.add)
                    nc.vector.tensor_copy(r1r[:, opb, :], xr1[:, opb, :])
                if debug:
                    nc.sync.dma_start(dbg["d_r1"][:], xr1[:, 0, :])
            es_attn.close()

        # ---------------- phase E: LN2 stats over own tokens --------------
        stat2 = poolR1.tile([4, OWN], f32r, name="stat2")
        rstd2_bc = poolR1.tile([128, OWN], f32, name="rstd2_bc")
        with ExitStack() as es_e:
            pe = es_e.enter_context(tc.tile_pool(name="pe_sb", bufs=1))
            pdram2 = es_e.enter_context(
                tc.tile_pool(name="pdram2", bufs=1, space="DRAM"))
            pe_ps = es_e.enter_context(
                tc.tile_pool(name="pe_ps", bufs=2, space="PSUM"))
            sq2s = []
            for kb in range(12):
                sq2 = pe.tile([128, OWN], bf16, tag="sq2", name=f"sq2_{kb}", bufs=12)
                nc.scalar.activation(sq2[:], r1r[:, kb, :], AF.Square)
                sq2s.append(sq2)
            mu2_ps = pe_ps.tile([2, OWN], f32, tag="mu2", name="mu2")
            s2_ps = pe_ps.tile([1, OWN], f32, tag="s2", name="s2")
            for kb in range(12):
                lhs = t_ones_ab[:, 0:2] if kb < 6 else t_ones_ab[:, 2:4]
                nc.tensor.matmul(mu2_ps[:], lhs, r1r[:, kb, :],
                                 start=(kb == 0), stop=(kb == 11))
                nc.tensor.matmul(s2_ps[:], t_ones_s[:], sq2s[kb][:],
                                 start=(kb == 0), stop=(kb == 11))
            mu2_sb = pe.tile([2, OWN], f32, tag="emusb", name="emusb")
            mu2_fl = pe.tile([1, 2, OWN], f32, tag="emufl", name="emufl")
            var = pe.tile([1, OWN], f32, tag="evar", name="evar")
            lnv = pe.tile([1, OWN], f32, tag="elnv", name="elnv")
            s2_c = pe.tile([1, OWN], f32, tag="es2c", name="es2c")
            nc.vector.tensor_scalar(mu2_sb[:], mu2_ps[:], 1.0 / C, None,
                                    op0=ALU.mult)
            nc.vector.tensor_scalar(s2_c[:], s2_ps[:], 1.0 / C, None,
                                    op0=ALU.mult)
            nc.sync.dma_start(mu2_fl[:, 0, :], mu2_sb[0:1, :])
            nc.sync.dma_start(mu2_fl[:, 1, :], mu2_sb[1:2, :])
            sq_mu2 = pe.tile([1, 2, OWN], f32, tag="esqmu", name="esqmu")
            nc.vector.tensor_tensor(sq_mu2[:], mu2_fl[:], mu2_fl[:], op=ALU.mult)
            nc.vector.tensor_tensor(var[:], s2_c[:], sq_mu2[:, 0, :],
                                    op=ALU.subtract)
            nc.vector.tensor_tensor(var[:], var[:], sq_mu2[:, 1, :],
                                    op=ALU.subtract)
            nc.scalar.activation(lnv[:], var[:], AF.Ln, bias=t_eps[:])
            nc.vector.tensor_copy(stat2[0:2, :], mu2_sb[:])
            std2_row = pe.tile([1, OWN], f32r, tag="estd", name="estd")
            nc.scalar.activation(std2_row[:], lnv[:], AF.Exp, scale=0.5)
            nc.sync.dma_start(stat2[2:3, :], std2_row[:])
            rstd2_row = pe.tile([1, OWN], f32r, tag="ers", name="ers")
            nc.scalar.activation(rstd2_row[:], lnv[:], AF.Exp, scale=-0.5)
            rstd2_dram = pdram2.tile([1, OWN], f32, name="rstd2_dram")
            nc.sync.dma_start(rstd2_dram[:], rstd2_row[:].bitcast(f32))
            bcast2 = bass.AP(tensor=rstd2_dram.tensor, offset=rstd2_dram[:].offset,
                             ap=[[0, 128]] + rstd2_dram[:].ap[1:])
            nc.sync.dma_start(rstd2_bc[:], bcast2)
            if debug:
                nc.sync.dma_start(dbg["d_stat2"][:], stat2[:].bitcast(f32))

        # ---------------- phase F: MLP, single 512-token pass -------------
        with ExitStack() as es_f:
            ph = es_f.enter_context(tc.tile_pool(name="ph", bufs=1))
            pw1 = es_f.enter_context(tc.tile_pool(name="pw1", bufs=3))
            pw2 = es_f.enter_context(tc.tile_pool(name="pw2", bufs=3))
            pscf = es_f.enter_context(tc.tile_pool(name="pscf", bufs=4))
            pout = es_f.enter_context(tc.tile_pool(name="pout", bufs=2))
            ps_f = es_f.enter_context(
                tc.tile_pool(name="ps_f", bufs=4, space="PSUM"))
            h_t = ph.tile([128, 48, OWN], bf16, name="h_t")
            for Cb in range(24):
                w1r_t = pw1.tile([128, 6, 128], bf16, tag="w1r",
                                 name=f"w1r{Cb}")
                w1i_t = pw1.tile([128, 6, 128], bf16, tag="w1i",
                                 name=f"w1i{Cb}")
                w1in_t = pw1.tile([128, 6, 128], bf16, tag="w1in",
                                  name=f"w1in{Cb}")
                w1s_t = pw1.tile([4, 256], f32r, tag="w1s",
                                 name=f"w1s{Cb}")
                nc.sync.dma_start(w1r_t[:], w_fc1r[Cb])
                nc.sync.dma_start(w1i_t[:], w_fc1i[Cb])
                nc.sync.dma_start(w1in_t[:], w_fc1in[Cb])
                nc.sync.dma_start(w1s_t[:], w_fc1_s[Cb])
                hr_ps = ps_f.tile([128, OWN], f32, tag="fps",
                                  name=f"hrps{Cb}")
                hi_ps = ps_f.tile([128, OWN], f32, tag="fps",
                                  name=f"hips{Cb}")
                for kb in range(6):
                    st = (kb == 0)
                    nc.tensor.matmul(hr_ps[:], w1r_t[:, kb, :],
                                     r1r[:, kb, :], start=st, stop=False)
                    nc.tensor.matmul(hi_ps[:], w1i_t[:, kb, :],
                                     r1r[:, kb, :], start=st, stop=False)
                for kb in range(6):
                    nc.tensor.matmul(hr_ps[:], w1in_t[:, kb, :],
                                     r1r[:, 6 + kb, :], start=False,
                                     stop=False)
                    nc.tensor.matmul(hi_ps[:], w1r_t[:, kb, :],
                                     r1r[:, 6 + kb, :], start=False,
                                     stop=False)
                nc.tensor.matmul(hr_ps[:], w1s_t[:, 0:128], stat2[:],
                                 start=False, stop=True)
                nc.tensor.matmul(hi_ps[:], w1s_t[:, 128:256], stat2[:],
                                 start=False, stop=True)
                gr = pscf.tile([128, OWN], f32, tag="g", name=f"gr{Cb}")
                gi = pscf.tile([128, OWN], f32, tag="g", name=f"gi{Cb}")
                nc.vector.tensor_tensor(gr[:], hr_ps[:], rstd2_bc[:],
                                        op=ALU.mult)
                nc.vector.tensor_tensor(gi[:], hi_ps[:], rstd2_bc[:],
                                        op=ALU.mult)
                nc.scalar.activation(h_t[:, Cb, :], gr[:], AF.Gelu)
                nc.scalar.activation(h_t[:, 24 + Cb, :], gi[:], AF.Gelu)
            if debug:
                h_dbg = pscf.tile([128, OWN], f32, tag="hdbg", name="hdbg")
                nc.vector.tensor_copy(h_dbg[:], h_t[:, 0, :])
                nc.sync.dma_start(dbg["d_h"][:], h_dbg[:])
            for j in range(6):
                w2r_t = pw2.tile([128, 24, 128], bf16, tag="w2r",
                                 name=f"w2r{j}")
                w2i_t = pw2.tile([128, 24, 128], bf16, tag="w2i",
                                 name=f"w2i{j}")
                w2in_t = pw2.tile([128, 24, 128], bf16, tag="w2in",
                                  name=f"w2in{j}")
                w2s_t = pw2.tile([4, 256], f32r, tag="w2s",
                                 name=f"w2s{j}")
                nc.sync.dma_start(w2r_t[:], w_fc2r[j])
                nc.sync.dma_start(w2i_t[:], w_fc2i[j])
                nc.sync.dma_start(w2in_t[:], w_fc2in[j])
                nc.sync.dma_start(w2s_t[:], w_fc2_s[j])
                or_ps = ps_f.tile([128, OWN], f32, tag="fps",
                                  name=f"orps{j}")
                oi_ps = ps_f.tile([128, OWN], f32, tag="fps",
                                  name=f"oips{j}")
                for kb in range(24):
                    st = (kb == 0)
                    nc.tensor.matmul(or_ps[:], w2r_t[:, kb, :], h_t[:, kb, :],
                                     start=st, stop=False)
                    nc.tensor.matmul(oi_ps[:], w2i_t[:, kb, :], h_t[:, kb, :],
                                     start=st, stop=False)
                for kb in range(24):
                    nc.tensor.matmul(or_ps[:], w2in_t[:, kb, :],
                                     h_t[:, 24 + kb, :],
                                     start=False, stop=False)
                    nc.tensor.matmul(oi_ps[:], w2r_t[:, kb, :],
                                     h_t[:, 24 + kb, :],
                                     start=False, stop=False)
                nc.tensor.matmul(or_ps[:], w2s_t[:, 0:128],
                                 t_stat_one[:], start=False, stop=True)
                nc.tensor.matmul(oi_ps[:], w2s_t[:, 128:256],
                                 t_stat_one[:], start=False, stop=True)
                o_r = pout.tile([128, OWN], f32, tag="o", name=f"or{j}")
                o_i = pout.tile([128, OWN], f32, tag="o", name=f"oi{j}")
                nc.vector.tensor_tensor(o_r[:], or_ps[:], xr1[:, j, :],
                                        op=ALU.add)
                nc.vector.tensor_tensor(o_i[:], oi_ps[:], xr1[:, 6 + j, :],
                                        op=ALU.add)
                nc.sync.dma_start(out_fm[j], o_r[:])
                nc.sync.dma_start(out_fm[6 + j], o_i[:])
    nc.compile()
    return nc


# --------------------------------------------------------------------------
# host side
# --------------------------------------------------------------------------

def _cx(a):
    return a[..., 0].astype(np.float64) + 1j * a[..., 1].astype(np.float64)


def _kcols(Wp, wsum, wb, plane, scale=1.0):
    """K-profile [1539, m] for output features with complex weight rows Wp
    [m, 768], LN fold sums wsum [m], bias-column wb [m]. K rows: xr(768),
    xi(768), mu_r, mu_i, std."""
    m = Wp.shape[0]
    out = np.zeros((1539, m), np.float64)
    if plane == "r":
        out[0:768] = Wp.real.T
        out[768:1536] = -Wp.imag.T
        out[1536] = -wsum.real
        out[1537] = wsum.imag
        out[1538] = wb.real
    else:
        out[0:768] = Wp.imag.T
        out[768:1536] = Wp.real.T
        out[1536] = -wsum.imag
        out[1537] = -wsum.real
        out[1538] = wb.imag
    return out * scale


def _bf(a):
    return np.ascontiguousarray(a).astype(BF16)


F8 = ml_dtypes.float8_e4m3


def _f8(a):
    return np.ascontiguousarray(a).astype(F8)


def _pmajor(a):
    """[12or6or24 kb, 128, n] -> [128, kb, n] partition-major contiguous."""
    return np.ascontiguousarray(np.transpose(a, (1, 0, 2)))


def _prep_weights(inputs):
    n1 = _cx(inputs["n1_w"]); b1 = _cx(inputs["n1_b"])
    n2 = _cx(inputs["n2_w"]); b2 = _cx(inputs["n2_b"])
    Wqkv = _cx(inputs["qkv_w"])          # [2304, 768]
    Wp = _cx(inputs["proj_w"])           # [768, 768]
    bp = _cx(inputs["proj_b"])           # [768]
    W1 = _cx(inputs["fc1_w"])            # [3072, 768]
    bf1 = _cx(inputs["fc1_b"])           # [3072]
    W2 = _cx(inputs["fc2_w"])            # [768, 3072]
    bf2 = _cx(inputs["fc2_b"])           # [768]

    d = {}
    # ---- qkv (LN1-folded) ----
    Wq, Wk, Wv = Wqkv[0:768], Wqkv[768:1536], Wqkv[1536:2304]

    def fold1(W):
        Wf = W * n1[None, :]
        return Wf, Wf.sum(1), W @ b1

    w_qkv = np.zeros((H, 128, 12, 384), BF16)
    w_qkv_s = np.zeros((H, 4, 384), np.float32)
    for h in range(H):
        rows = slice(h * DH, (h + 1) * DH)
        Qf, Qs, Qb = fold1(Wq[rows])
        Kf, Ks, Kb_ = fold1(Wk[rows])
        q1 = np.hstack([_kcols(Qf, Qs, Qb, "r", SCALE),
                        _kcols(Qf, Qs, Qb, "i", -SCALE)])
        q3 = np.hstack([_kcols(Qf, Qs, Qb, "i", SCALE),
                        _kcols(Qf, Qs, Qb, "r", SCALE)])
        kk = np.hstack([_kcols(Kf, Ks, Kb_, "r"), _kcols(Kf, Ks, Kb_, "i")])
        blk = np.hstack([q1, q3, kk]).astype(np.float32)       # [1539, 384]
        w_qkv[h] = _bf(_pmajor(blk[0:1536].reshape(12, 128, 384)))
        w_qkv_s[h, 0:3] = blk[1536:1539]
    d["w_qkv"] = w_qkv
    d["w_qkv_s"] = round_fp32r(w_qkv_s)

    # ---- v (LN1-folded), rhs layout; cols: pair*256+slot*128+plane*64+dh
    wv_full = np.zeros((1539, 1536), np.float64)
    for h in range(H):
        rows = slice(h * DH, (h + 1) * DH)
        Vf, Vs, Vb = fold1(Wv[rows])
        base = h * 128
        wv_full[:, base:base + 64] = _kcols(Vf, Vs, Vb, "r")
        wv_full[:, base + 64:base + 128] = _kcols(Vf, Vs, Vb, "i")
    w_v = np.zeros((6, 128, 12, 256), BF16)
    for pair in range(6):
        csl = slice(pair * 256, pair * 256 + 256)
        w_v[pair] = _bf(_pmajor(wv_full[0:1536, csl].reshape(12, 128, 256)))
    d["w_v"] = w_v
    wvs = np.zeros((4, 1536), np.float32)
    wvs[0:3] = wv_full[1536:1539]
    d["w_v_s"] = round_fp32r(wvs)

    # ---- proj (plain + bias); K rows = attn features: per head [a_r(64); a_i(64)]
    w_proj = np.zeros((12, 128, 12, 128), BF16)
    w_proj_s = np.zeros((12, 4, 128), np.float32)
    for opb in range(12):
        plane = "r" if opb < 6 else "i"
        orow = slice((opb % 6) * 128, (opb % 6) * 128 + 128)
        Wpo = Wp[orow]                               # [128, 768] complex
        prof = np.zeros((1536, 128), np.float64)
        for hh in range(H):
            cols = slice(hh * DH, (hh + 1) * DH)
            if plane == "r":
                prof[hh * 128:hh * 128 + 64] = Wpo.real[:, cols].T
                prof[hh * 128 + 64:hh * 128 + 128] = -Wpo.imag[:, cols].T
            else:
                prof[hh * 128:hh * 128 + 64] = Wpo.imag[:, cols].T
                prof[hh * 128 + 64:hh * 128 + 128] = Wpo.real[:, cols].T
        w_proj[opb] = _bf(_pmajor(prof.reshape(12, 128, 128)))
        w_proj_s[opb, 0] = (bp.real if plane == "r" else bp.imag)[orow]
    d["w_proj"] = w_proj
    d["w_proj_s"] = round_fp32r(w_proj_s)

    # ---- fc1 (LN2-folded, shared-tile form) ----
    W1f = W1 * n2[None, :]
    W1s = W1f.sum(1)
    W1b = W1 @ b2 + bf1
    w_fc1r = np.zeros((24, 128, 6, 128), BF16)
    w_fc1i = np.zeros((24, 128, 6, 128), BF16)
    w_fc1in = np.zeros((24, 128, 6, 128), BF16)
    w_fc1_s = np.zeros((24, 4, 256), np.float32)
    for Cb in range(24):
        orow = slice(Cb * 128, (Cb + 1) * 128)
        tr = np.zeros((6, 128, 128), np.float64)
        ti = np.zeros((6, 128, 128), np.float64)
        for kb in range(6):
            icol = slice(kb * 128, (kb + 1) * 128)
            tr[kb] = W1f.real[orow, icol].T
            ti[kb] = W1f.imag[orow, icol].T
        w_fc1r[Cb] = _bf(_pmajor(tr))
        w_fc1i[Cb] = _bf(_pmajor(ti))
        w_fc1in[Cb] = _bf(_pmajor(-ti))
        w_fc1_s[Cb, 0, 0:128] = -W1s.real[orow]
        w_fc1_s[Cb, 1, 0:128] = W1s.imag[orow]
        w_fc1_s[Cb, 2, 0:128] = W1b.real[orow]
        w_fc1_s[Cb, 0, 128:256] = -W1s.imag[orow]
        w_fc1_s[Cb, 1, 128:256] = -W1s.real[orow]
        w_fc1_s[Cb, 2, 128:256] = W1b.imag[orow]
    d["w_fc1r"] = w_fc1r
    d["w_fc1i"] = w_fc1i
    d["w_fc1in"] = w_fc1in
    d["w_fc1_s"] = round_fp32r(w_fc1_s)

    # ---- fc2 (plain + bias) ----
    w_fc2r = np.zeros((6, 128, 24, 128), BF16)
    w_fc2i = np.zeros((6, 128, 24, 128), BF16)
    w_fc2in = np.zeros((6, 128, 24, 128), BF16)
    w_fc2_s = np.zeros((6, 4, 256), np.float32)
    for j in range(6):
        orow = slice(j * 128, (j + 1) * 128)
        tr = np.zeros((24, 128, 128), np.float64)
        ti = np.zeros((24, 128, 128), np.float64)
        for kb in range(24):
            icol = slice(kb * 128, (kb + 1) * 128)
            tr[kb] = W2.real[orow, icol].T
            ti[kb] = W2.imag[orow, icol].T
        w_fc2r[j] = _bf(_pmajor(tr))
        w_fc2i[j] = _bf(_pmajor(ti))
        w_fc2in[j] = _bf(_pmajor(-ti))
        w_fc2_s[j, 0, 0:128] = bf2.real[orow]
        w_fc2_s[j, 0, 128:256] = bf2.imag[orow]
    d["w_fc2r"] = w_fc2r
    d["w_fc2i"] = w_fc2i
    d["w_fc2in"] = w_fc2in
    d["w_fc2_s"] = round_fp32r(w_fc2_s)

    # ---- consts ----
    d["ones_col"] = np.ones((128, 1), BF16)
    oab = np.zeros((128, 4), np.float32)
    oab[:, 0] = 1.0
    oab[:, 3] = 1.0
    d["ones_ab"] = oab.astype(BF16)
    d["ones_s"] = np.ones((128, 1), BF16)
    so = np.zeros((4, OWN), np.float32)
    so[0] = 1.0
    d["stat_one"] = so
    d["ident8"] = np.eye(8, dtype=np.float32)
    return d


_NC_CACHE = {}


def kernel(**inputs):
    debug = bool(inputs.pop("_debug", False))
    if debug not in _NC_CACHE:
        _NC_CACHE[debug] = build_nc(debug=debug)
    nc = _NC_CACHE[debug]

    shared = _prep_weights(inputs)
    x = np.asarray(inputs["x"], np.float32)          # [B, N, C, 2]

    in_maps = []
    for c in range(NCORES):
        b, half = divmod(c, 2)
        xr_ = x[b, :, :, 0].T                        # [768, 1024]
        xi_ = x[b, :, :, 1].T
        stack = np.concatenate([xr_, xi_], 0)        # [1536, 1024]
        if half == 1:
            stack = np.concatenate([stack[:, OWN:], stack[:, :OWN]], 1)
        m = dict(shared)
        m["x_r"] = np.ascontiguousarray(
            stack.reshape(12, 128, N).transpose(1, 0, 2)).astype(BF16)
        m["x_own"] = np.ascontiguousarray(
            stack[:, 0:OWN].reshape(12, 128, OWN).transpose(1, 0, 2))
        in_maps.append(m)

    res = run_bass_kernel_spmd(nc, in_maps, list(range(NCORES)))
    out = np.empty((B, N, C, 2), np.float32)
    for c in range(NCORES):
        b, half = divmod(c, 2)
        o = res.results[c]["out_fm"]                 # [12, 128, OWN]
        sl = slice(half * OWN, half * OWN + OWN)
        out[b, sl, :, 0] = o[0:6].reshape(768, OWN).T
        out[b, sl, :, 1] = o[6:12].reshape(768, OWN).T
    if debug:
        return out, res
    return out
